# revision 12
# baseline (speedup 1.0000x reference)
"""Trainium2 Bass kernel for nn_ConstellationRelay.

Computation (per token, D=1024, A=16 anchors, C=8 comps, dc=64):
  h   = l2norm(layernorm(x; ln_g, ln_b))
  tri = 1 - h @ l2norm(anchors).T                       (N, 16)
  u   = relu(einsum('nak,kae->nke', tri_g, W1) + b1)^2  (N, 8, 128)
  y   = layernorm_c(u @ W2 + b2; cg, cb)                (N, 8, 64)
  out = x + sigmoid(gate) * (y.flat @ Wp + bp)

Strategy: pure data-parallel over batch (one of 8 NeuronCores per batch row).
On-device fast path requires ln_g==1, ln_b==0 (always true for this problem's
setup_inputs); every other parameter is handled generally via host-side
folding:
  * h = (x - mu)/sqrt(1024*var)  -- eps cancels exactly through the l2norm
  * tri/W1 stage folded into two small matmuls (A0 = a_norm @ h; expand with
    -W1exp and bias (sum_m W1exp + b1) applied in the ReLU activation)
  * comp-LN mean-subtraction folded into centered W2/b2 (host)
  * cg, cb, bp, sigmoid(gate) folded into Wp/const (host)
Layout: token-major for stats/residual, feature-major (via DMA-transpose of
bf16 h) for all matmuls; proj matmul operand-swapped so the residual add
lands token-major in PSUM.
"""

import functools
import os
import sys

import numpy as np

for _p in ("/opt/trn_rl_repo",):
    if _p not in sys.path and os.path.isdir(_p):
        sys.path.insert(0, _p)

B, S, D = 8, 4096, 1024
A, C, DC = 16, 8, 64
APC = A // C  # anchors per compartment
E2 = 2 * DC  # 128, expanded width per comp
NCORES = 8
TOK = 512  # tokens per pipeline tile
NTILE = S // TOK  # 8
NCH = TOK // 128  # 4 token chunks of 128 per tile
KD = D // 128  # 8 feature chunks


def _np_reference(x, anchors, ln_g, ln_b, W1, b1, W2, b2, cg, cb, Wp, bp, gate):
    """Pure-numpy fallback, mirrors reference.py (used only if ln_g/ln_b
    deviate from the values this problem's setup_inputs produces)."""
    x = x.astype(np.float32)
    N = x.shape[0] * x.shape[1]
    xf = x.reshape(N, D)
    mu = xf.mean(-1, keepdims=True)
    var = ((xf - mu) ** 2).mean(-1, keepdims=True)
    h = (xf - mu) / np.sqrt(var + 1e-5) * ln_g + ln_b
    h = h / np.maximum(np.linalg.norm(h, axis=-1, keepdims=True), 1e-12)
    a = anchors / np.maximum(np.linalg.norm(anchors, axis=-1, keepdims=True), 1e-12)
    tri = 1.0 - h @ a.T
    g = tri.reshape(N, APC, C)
    u = np.einsum("nak,kae->nke", g, W1) + b1
    u = np.square(np.maximum(u, 0.0))
    y = np.einsum("nke,ked->nkd", u, W2) + b2
    muy = y.mean(-1, keepdims=True)
    vy = ((y - muy) ** 2).mean(-1, keepdims=True)
    y = (y - muy) / np.sqrt(vy + 1e-5) * cg + cb
    upd = y.reshape(N, C * DC) @ Wp + bp
    sig = 1.0 / (1.0 + np.exp(-gate))
    return (xf + sig * upd).reshape(x.shape).astype(np.float32)


@functools.lru_cache(maxsize=4)
def _build_program(n_tokens=S, use_const=False, interleaved_t=True,
                   use_recip_approx=True):
    """Build + schedule the single-core Bass program (same program runs SPMD
    on all 8 cores).

    interleaved_t: if True, the 3D-output dma_start_transpose writes feature
    d of h to (partition=d//KD, sub=d%KD); host packs the anchor matrix to
    match.  If False, use 32 plain 128x128 transposes with the natural
    d=(chunk*128+p) layout.
    """
    import concourse.bacc as bacc
    import concourse.mybir as mybir
    import concourse.tile as tile

    f32 = mybir.dt.float32
    bf16 = mybir.dt.bfloat16
    AF = mybir.ActivationFunctionType
    OP = mybir.AluOpType

    ntile = n_tokens // TOK

    nc = bacc.Bacc("TRN2", target_bir_lowering=False, debug=False,
                   num_devices=NCORES)

    x_d = nc.dram_tensor("x", [n_tokens, D], f32, kind="ExternalInput")
    agt_d = nc.dram_tensor("agt", [128, KD, A], bf16, kind="ExternalInput")
    w1e_d = nc.dram_tensor("w1e", [A, KD, 128], bf16, kind="ExternalInput")
    biasu_d = nc.dram_tensor("biasu", [128, KD], f32, kind="ExternalInput")
    w2c_d = nc.dram_tensor("w2c", [128, C, DC], bf16, kind="ExternalInput")
    vstl_d = nc.dram_tensor("vstl", [128, 4, C], bf16, kind="ExternalInput")
    b2f_d = nc.dram_tensor("b2f", [128, 4], f32, kind="ExternalInput")
    wpf_d = nc.dram_tensor("wpf", [128, 4, 2, 512], bf16, kind="ExternalInput")
    cvec_d = nc.dram_tensor("cvec", [1, 2, 512], bf16, kind="ExternalInput") \
        if use_const else None
    out_d = nc.dram_tensor("out", [n_tokens, D], f32, kind="ExternalOutput")

    from contextlib import ExitStack

    with tile.TileContext(nc) as tc, ExitStack() as ctx:
        pp = ctx.enter_context(tc.tile_pool(name="params", bufs=1))
        agt = pp.tile([128, KD, A], bf16)
        nc.sync.dma_start(out=agt, in_=agt_d[:, :, :])
        w1e = pp.tile([48, KD, 128], bf16)
        nc.sync.dma_start(out=w1e[0:A, :, :], in_=w1e_d[:, :, :])
        nc.sync.dma_start(out=w1e[32:32 + A, :, :], in_=w1e_d[:, :, :])
        biasu = pp.tile([128, KD], f32)
        nc.sync.dma_start(out=biasu, in_=biasu_d[:, :])
        w2c = pp.tile([128, C, DC], bf16)
        nc.sync.dma_start(out=w2c, in_=w2c_d[:, :, :])
        vstl = pp.tile([128, 4, C], bf16)
        nc.sync.dma_start(out=vstl, in_=vstl_d[:, :, :])
        b2f = pp.tile([128, 4], f32)
        nc.sync.dma_start(out=b2f, in_=b2f_d[:, :])
        wpf = pp.tile([128, 4, 2, 512], bf16)
        nc.sync.dma_start(out=wpf, in_=wpf_d[:, :, :, :])
        if use_const:
            cvec = pp.tile([1, 2, 512], bf16)
            nc.sync.dma_start(out=cvec, in_=cvec_d[:, :, :])
            ones1 = pp.tile([1, 128], bf16)
            nc.vector.memset(ones1, 1.0)
        ctiny = pp.tile([128, 1], f32)
        nc.vector.memset(ctiny, 1e-38)
        ceps = pp.tile([C, 1], f32)
        nc.vector.memset(ceps, 1e-5)

        px = ctx.enter_context(tc.tile_pool(name="px", bufs=2))
        psm = ctx.enter_context(tc.tile_pool(name="psm", bufs=8))
        # PSUM pools: 2 + 2 + 4 = 8 banks exactly.
        ps_small = ctx.enter_context(tc.tile_pool(name="ps_small", bufs=2,
                                                  space="PSUM"))
        ps_y = ctx.enter_context(tc.tile_pool(name="ps_y", bufs=2,
                                              space="PSUM"))
        ps_mm = ctx.enter_context(tc.tile_pool(name="ps_mm", bufs=4,
                                               space="PSUM"))
        pdram = ctx.enter_context(tc.tile_pool(name="pdram", bufs=2,
                                               space="DRAM"))

        for t in range(ntile):
            row0 = t * TOK
            xt = px.tile([128, NCH, D], f32, tag="xt")
            for cch in range(NCH):
                nc.sync.dma_start(
                    out=xt[:, cch, :],
                    in_=x_d[row0 + cch * 128: row0 + (cch + 1) * 128, :])

            # --- token-major stats + normalize-cast -----------------------
            hb = px.tile([128, NCH, D], bf16, tag="hb")
            mv = psm.tile([128, NCH, 2], f32, tag="mv")
            for cch in range(NCH):
                st = psm.tile([128, 2, 6], f32, tag="st")
                xr = xt[:, cch, :].rearrange("p (s f) -> p s f", s=2)
                nc.vector.bn_stats(out=st[:, 0, :], in_=xr[:, 0, :])
                nc.vector.bn_stats(out=st[:, 1, :], in_=xr[:, 1, :])
                nc.vector.bn_aggr(out=mv[:, cch, :], in_=st)
            sd = psm.tile([128, NCH], f32, tag="sd")
            # sd = sqrt(1024*var); e = 1/sd; bias = -mu*e
            nc.scalar.activation(sd, mv[:, :, 1], AF.Sqrt, bias=ctiny,
                                 scale=float(D))
            ee = psm.tile([128, NCH], f32, tag="ee")
            nc.vector.reciprocal(ee, sd)
            bh = psm.tile([128, NCH], f32, tag="bh")
            nc.vector.scalar_tensor_tensor(
                out=bh, in0=mv[:, :, 0], scalar=-1.0, in1=ee,
                op0=OP.mult, op1=OP.mult)
            for cch in range(NCH):
                nc.scalar.activation(hb[:, cch, :], xt[:, cch, :], AF.Identity,
                                     bias=bh[:, cch:cch + 1],
                                     scale=ee[:, cch:cch + 1])

            # --- transpose h to feature-major -----------------------------
            hbT = px.tile([128, KD, TOK], bf16, tag="hbT", bufs=3)
            if interleaved_t:
                for cch in range(NCH):
                    nc.sync.dma_start_transpose(
                        out=hbT[:, :, cch * 128:(cch + 1) * 128],
                        in_=hb[:, cch, :])
            else:
                for cch in range(NCH):
                    for dch in range(KD):
                        nc.sync.dma_start_transpose(
                            out=hbT[:, dch, cch * 128:(cch + 1) * 128],
                            in_=hb[:, cch, dch * 128:(dch + 1) * 128])

            # --- A0 = a_norm @ h  [16, TOK] -------------------------------
            a0p = ps_small.tile([A, TOK], f32, tag="small")
            for dch in range(KD):
                nc.tensor.matmul(a0p, lhsT=agt[:, dch, :], rhs=hbT[:, dch, :],
                                 start=(dch == 0), stop=(dch == KD - 1))
            a0 = psm.tile([48, TOK], bf16, tag="a0", bufs=2)
            nc.vector.tensor_copy(out=a0[0:A, :], in_=a0p)
            nc.sync.dma_start(out=a0[32:32 + A, :], in_=a0[0:A, :])

            # --- expand: u_pre = -W1exp.T @ A0 ; r = relu(u_pre + biasu) --
            rbig = px.tile([128, KD, TOK], bf16, tag="rbig")
            for kg in range(KD // 2):
                k0, k1 = 2 * kg, 2 * kg + 1
                up0 = ps_mm.tile([128, TOK], f32, tag="mmout")
                nc.tensor.matmul(up0, lhsT=w1e[0:A, k0, :], rhs=a0[0:A, :],
                                 start=True, stop=True)
                up1 = ps_mm.tile([128, TOK], f32, tag="mmout")
                nc.tensor.matmul(up1, lhsT=w1e[32:32 + A, k1, :],
                                 rhs=a0[32:32 + A, :], start=True, stop=True)
                nc.scalar.activation(rbig[:, k0, :], up0, AF.Relu,
                                     bias=biasu[:, k0:k0 + 1], scale=1.0)
                nc.scalar.activation(rbig[:, k1, :], up1, AF.Relu,
                                     bias=biasu[:, k1:k1 + 1], scale=1.0)
            ubig = px.tile([128, KD, TOK], bf16, tag="ubig", bufs=3)
            nc.gpsimd.tensor_mul(
                ubig.rearrange("p a b -> p (a b)"),
                rbig.rearrange("p a b -> p (a b)"),
                rbig.rearrange("p a b -> p (a b)"))

            # --- comp matmul + centered bias ------------------------------
            yb = px.tile([128, 4, TOK], bf16, tag="yb")
            for j in range(4):
                yp = ps_y.tile([128, TOK], f32, tag="ypre")
                nc.tensor.matmul(yp[0:64, :], lhsT=w2c[:, 2 * j, :],
                                 rhs=ubig[:, 2 * j, :], start=True, stop=True)
                nc.tensor.matmul(yp[64:128, :], lhsT=w2c[:, 2 * j + 1, :],
                                 rhs=ubig[:, 2 * j + 1, :], start=True,
                                 stop=True, tile_position=(0, 64))
                nc.scalar.activation(yb[:, j, :], yp, AF.Identity,
                                     bias=b2f[:, j:j + 1], scale=1.0)
            sqy = px.tile([128, 4, TOK], bf16, tag="sqy")
            nc.gpsimd.tensor_mul(
                sqy.rearrange("p a b -> p (a b)"),
                yb.rearrange("p a b -> p (a b)"),
                yb.rearrange("p a b -> p (a b)"))

            # --- per-comp variance via PE, rstd, broadcast ----------------
            vst = ps_small.tile([C, TOK], f32, tag="small")
            for j in range(4):
                nc.tensor.matmul(vst, lhsT=vstl[:, j, :], rhs=sqy[:, j, :],
                                 start=(j == 0), stop=(j == 3))
            sd2 = psm.tile([C, TOK], f32, tag="sd2", bufs=2)
            nc.scalar.activation(sd2, vst, AF.Sqrt, bias=ceps, scale=1.0)
            rr = psm.tile([C, TOK], f32, tag="rr", bufs=2)
            if use_recip_approx:
                nc.vector.reciprocal_approx_fast(out=rr, in_=sd2)
            else:
                nc.vector.reciprocal(out=rr, in_=sd2)
            rrb = psm.tile([C, TOK], bf16, tag="rrb", bufs=2)
            nc.vector.tensor_copy(out=rrb, in_=rr)
            rrd = pdram.tile([C, TOK], bf16, tag="rrd")
            nc.sync.dma_start(out=rrd, in_=rrb)
            rbB = px.tile([128, 4, TOK], bf16, tag="rbB")
            import concourse.bass as bass_mod
            for p1 in range(2):
                src = bass_mod.AP(
                    tensor=rrd.tensor if hasattr(rrd, 'tensor') else rrd,
                    offset=(rrd.offset if hasattr(rrd, 'offset') else 0)
                    + p1 * TOK,
                    ap=[[0, 64], [2 * TOK, 4], [1, TOK]])
                nc.sync.dma_start(out=rbB[64 * p1:64 * (p1 + 1), :, :],
                                  in_=src)

            ycT = px.tile([128, 4, TOK], bf16, tag="ycT")
            for j in range(4):
                nc.vector.tensor_mul(ycT[:, j, :], yb[:, j, :], rbB[:, j, :])

            # --- proj (operand-swapped -> token-major) + residual ---------
            for cch in range(NCH):
                osb = px.tile([128, D], f32, tag="osb", bufs=3)
                for hf in range(2):
                    ud = ps_mm.tile([128, 512], f32, tag="mmout")
                    for j in range(4):
                        nc.tensor.matmul(
                            ud, lhsT=ycT[:, j, cch * 128:(cch + 1) * 128],
                            rhs=wpf[:, j, hf, :],
                            start=(j == 0),
                            stop=(j == 3 and not use_const))
                    if use_const:
                        nc.tensor.matmul(ud, lhsT=ones1, rhs=cvec[:, hf, :],
                                         start=False, stop=True)
                    nc.vector.tensor_add(
                        osb[:, hf * 512:(hf + 1) * 512], ud,
                        xt[:, cch, hf * 512:(hf + 1) * 512])
                nc.sync.dma_start(
                    out=out_d[row0 + cch * 128: row0 + (cch + 1) * 128, :],
                    in_=osb[:, :])

    nc.compile()
    return nc


def _pack_params(anchors, ln_g, W1, b1, W2, b2, cg, cb, Wp, bp, gate,
                 interleaved_t=True):
    f32 = np.float32
    anchors = anchors.astype(f32)
    an = anchors / np.maximum(
        np.linalg.norm(anchors.astype(np.float64), axis=1, keepdims=True),
        1e-12).astype(f32)
    ag = (an * ln_g[None, :].astype(f32)).astype(f32)  # [A, D]

    # agt[p, s, m] = ag[m, d(p,s)]
    agt = np.zeros((128, KD, A), f32)
    dd = np.arange(D)
    if interleaved_t:
        pidx, sidx = dd // KD, dd % KD
    else:
        pidx, sidx = dd % 128, dd // 128
    agt[pidx, sidx, :] = ag.T[dd, :]

    # W1exp[m, f] with m=j*C+k2, f=k*128+e -> value W1[k, j, e] iff k2==k
    W1 = W1.astype(f32)
    w1exp = np.zeros((A, C, E2), f32)
    for m in range(A):
        j, k2 = m // C, m % C
        w1exp[m, k2, :] = W1[k2, j, :]
    w1e = (-w1exp).reshape(A, C, E2)  # [16, 8, 128] (f = k*128+e)
    sf = w1exp.sum(axis=0)  # [C, E2]
    biasu = (sf + b1.astype(f32)).T.copy()  # [128, C] (partition=e, col=k)

    W2 = W2.astype(f32)
    w2m = W2.mean(axis=2, keepdims=True)
    w2cent = W2 - w2m  # [C, E2, DC]
    w2c = np.transpose(w2cent, (1, 0, 2)).copy()  # [128, C, 64]
    b2c = b2.astype(f32) - b2.astype(f32).mean(axis=1, keepdims=True)  # [C, DC]

    b2f = np.zeros((128, 4), f32)
    vstl = np.zeros((128, 4, C), f32)
    for j in range(4):
        for p in range(128):
            kk = 2 * j + p // 64
            b2f[p, j] = b2c[kk, p % 64]
            vstl[p, j, kk] = 1.0 / DC

    sig = (1.0 / (1.0 + np.exp(-gate.astype(np.float64)))).astype(f32)  # [D]
    wpfold = (cg.astype(f32).reshape(C * DC, 1) * Wp.astype(f32)) * sig[None, :]
    wpf = np.ascontiguousarray(
        wpfold.reshape(4, 128, 2, 512).transpose(1, 0, 2, 3))

    const = (cb.astype(f32).reshape(-1) @ Wp.astype(f32) + bp.astype(f32)) * sig
    use_const = bool(np.max(np.abs(const)) > 0)

    import ml_dtypes
    bf16 = ml_dtypes.bfloat16
    params = dict(
        agt=agt.astype(bf16),
        w1e=w1e.astype(bf16),
        biasu=biasu.astype(f32),
        w2c=w2c.astype(bf16),
        vstl=vstl.astype(bf16),
        b2f=b2f.astype(f32),
        wpf=wpf.astype(bf16),
    )
    if use_const:
        params["cvec"] = const.reshape(1, 2, 512).astype(bf16)
    return params, use_const


def kernel(**inputs):
    x = np.asarray(inputs["x"], dtype=np.float32)
    ln_g = np.asarray(inputs["ln_g"], dtype=np.float32)
    ln_b = np.asarray(inputs["ln_b"], dtype=np.float32)

    fast = (np.allclose(ln_g, 1.0, atol=1e-12) and
            np.allclose(ln_b, 0.0, atol=1e-12))
    if not fast:
        return _np_reference(
            x, *[np.asarray(inputs[k], dtype=np.float32) for k in
                 ("anchors", "ln_g", "ln_b", "W1", "b1", "W2", "b2", "cg",
                  "cb", "Wp", "bp", "gate")])

    params, use_const = _pack_params(
        inputs["anchors"], ln_g, inputs["W1"], inputs["b1"], inputs["W2"],
        inputs["b2"], inputs["cg"], inputs["cb"], inputs["Wp"], inputs["bp"],
        inputs["gate"], interleaved_t=INTERLEAVED_T)

    nc = _build_program(S, use_const, INTERLEAVED_T, USE_RECIP_APPROX)

    from concourse.bass_utils import run_bass_kernel_spmd
    in_maps = []
    for b in range(NCORES):
        m = dict(params)
        m["x"] = np.ascontiguousarray(x[b])
        in_maps.append(m)
    res = run_bass_kernel_spmd(nc, in_maps, core_ids=list(range(NCORES)))
    out = np.stack([res.results[b]["out"] for b in range(NCORES)], axis=0)
    return out.reshape(B, S, D).astype(np.float32)


INTERLEAVED_T = True
USE_RECIP_APPROX = True


# revision 13
# speedup vs baseline: 1.1906x; 1.1906x over previous
"""Trainium2 Bass kernel for nn_ConstellationRelay.

Computation (per token, D=1024, A=16 anchors, C=8 comps, dc=64):
  h   = l2norm(layernorm(x; ln_g, ln_b))
  tri = 1 - h @ l2norm(anchors).T                       (N, 16)
  u   = relu(einsum('nak,kae->nke', tri_g, W1) + b1)^2  (N, 8, 128)
  y   = layernorm_c(u @ W2 + b2; cg, cb)                (N, 8, 64)
  out = x + sigmoid(gate) * (y.flat @ Wp + bp)

Strategy: pure data-parallel over batch (one of 8 NeuronCores per batch row).
On-device fast path requires ln_g==1, ln_b==0 (always true for this problem's
setup_inputs); every other parameter is handled generally via host-side
folding:
  * h = (x - mu)/sqrt(1024*var)  -- eps cancels exactly through the l2norm
  * tri/W1 stage folded into two small matmuls (A0 = a_norm @ h; expand with
    -W1exp and bias (sum_m W1exp + b1) applied in the ReLU activation)
  * comp-LN mean-subtraction folded into centered W2/b2 (host)
  * cg, cb, bp, sigmoid(gate) folded into Wp/const (host)
Layout: token-major for stats/residual, feature-major (via DMA-transpose of
bf16 h) for all matmuls; proj matmul operand-swapped so the residual add
lands token-major in PSUM.
"""

import functools
import os
import sys

import numpy as np

for _p in ("/opt/trn_rl_repo",):
    if _p not in sys.path and os.path.isdir(_p):
        sys.path.insert(0, _p)

B, S, D = 8, 4096, 1024
A, C, DC = 16, 8, 64
APC = A // C  # anchors per compartment
E2 = 2 * DC  # 128, expanded width per comp
NCORES = 8
TOK = 512  # tokens per pipeline tile
NTILE = S // TOK  # 8
NCH = TOK // 128  # 4 token chunks of 128 per tile
KD = D // 128  # 8 feature chunks


def _np_reference(x, anchors, ln_g, ln_b, W1, b1, W2, b2, cg, cb, Wp, bp, gate):
    """Pure-numpy fallback, mirrors reference.py (used only if ln_g/ln_b
    deviate from the values this problem's setup_inputs produces)."""
    x = x.astype(np.float32)
    N = x.shape[0] * x.shape[1]
    xf = x.reshape(N, D)
    mu = xf.mean(-1, keepdims=True)
    var = ((xf - mu) ** 2).mean(-1, keepdims=True)
    h = (xf - mu) / np.sqrt(var + 1e-5) * ln_g + ln_b
    h = h / np.maximum(np.linalg.norm(h, axis=-1, keepdims=True), 1e-12)
    a = anchors / np.maximum(np.linalg.norm(anchors, axis=-1, keepdims=True), 1e-12)
    tri = 1.0 - h @ a.T
    g = tri.reshape(N, APC, C)
    u = np.einsum("nak,kae->nke", g, W1) + b1
    u = np.square(np.maximum(u, 0.0))
    y = np.einsum("nke,ked->nkd", u, W2) + b2
    muy = y.mean(-1, keepdims=True)
    vy = ((y - muy) ** 2).mean(-1, keepdims=True)
    y = (y - muy) / np.sqrt(vy + 1e-5) * cg + cb
    upd = y.reshape(N, C * DC) @ Wp + bp
    sig = 1.0 / (1.0 + np.exp(-gate))
    return (xf + sig * upd).reshape(x.shape).astype(np.float32)


@functools.lru_cache(maxsize=4)
def _build_program(n_tokens=S, use_const=False, interleaved_t=True,
                   use_recip_approx=True):
    """Build + schedule the single-core Bass program (same program runs SPMD
    on all 8 cores).

    interleaved_t: if True, the 3D-output dma_start_transpose writes feature
    d of h to (partition=d//KD, sub=d%KD); host packs the anchor matrix to
    match.  If False, use 32 plain 128x128 transposes with the natural
    d=(chunk*128+p) layout.
    """
    import concourse.bacc as bacc
    import concourse.mybir as mybir
    import concourse.tile as tile

    f32 = mybir.dt.float32
    bf16 = mybir.dt.bfloat16
    AF = mybir.ActivationFunctionType
    OP = mybir.AluOpType

    ntile = n_tokens // TOK

    nc = bacc.Bacc("TRN2", target_bir_lowering=False, debug=False,
                   num_devices=NCORES)

    x_d = nc.dram_tensor("x", [n_tokens, D], f32, kind="ExternalInput")
    agt_d = nc.dram_tensor("agt", [128, KD, A], bf16, kind="ExternalInput")
    w1e_d = nc.dram_tensor("w1e", [A, KD, 128], bf16, kind="ExternalInput")
    biasu_d = nc.dram_tensor("biasu", [128, KD], f32, kind="ExternalInput")
    w2c_d = nc.dram_tensor("w2c", [128, C, DC], bf16, kind="ExternalInput")
    vstl_d = nc.dram_tensor("vstl", [128, 4, C], bf16, kind="ExternalInput")
    b2f_d = nc.dram_tensor("b2f", [128, 4], f32, kind="ExternalInput")
    wpf_d = nc.dram_tensor("wpf", [128, 4, 2, 512], bf16, kind="ExternalInput")
    cvec_d = nc.dram_tensor("cvec", [1, 2, 512], bf16, kind="ExternalInput") \
        if use_const else None
    out_d = nc.dram_tensor("out", [n_tokens, D], f32, kind="ExternalOutput")

    from contextlib import ExitStack

    with tile.TileContext(nc) as tc, ExitStack() as ctx:
        pp = ctx.enter_context(tc.tile_pool(name="params", bufs=1))
        agt = pp.tile([128, KD, A], bf16)
        nc.sync.dma_start(out=agt, in_=agt_d[:, :, :])
        w1e = pp.tile([48, KD, 128], bf16)
        nc.sync.dma_start(out=w1e[0:A, :, :], in_=w1e_d[:, :, :])
        nc.sync.dma_start(out=w1e[32:32 + A, :, :], in_=w1e_d[:, :, :])
        biasu = pp.tile([128, KD], f32)
        nc.sync.dma_start(out=biasu, in_=biasu_d[:, :])
        w2c = pp.tile([128, C, DC], bf16)
        nc.sync.dma_start(out=w2c, in_=w2c_d[:, :, :])
        vstl = pp.tile([128, 4, C], bf16)
        nc.sync.dma_start(out=vstl, in_=vstl_d[:, :, :])
        b2f = pp.tile([128, 4], f32)
        nc.sync.dma_start(out=b2f, in_=b2f_d[:, :])
        wpf = pp.tile([128, 4, 2, 512], bf16)
        nc.sync.dma_start(out=wpf, in_=wpf_d[:, :, :, :])
        if use_const:
            cvec = pp.tile([1, 2, 512], bf16)
            nc.sync.dma_start(out=cvec, in_=cvec_d[:, :, :])
            ones1 = pp.tile([1, 128], bf16)
            nc.vector.memset(ones1, 1.0)
        ctiny = pp.tile([128, 1], f32)
        nc.vector.memset(ctiny, 1e-38)
        ceps = pp.tile([C, 1], f32)
        nc.vector.memset(ceps, 1e-5)
        czero = pp.tile([C, 1], f32)
        nc.vector.memset(czero, 0.0)

        px = ctx.enter_context(tc.tile_pool(name="px", bufs=2))
        psm = ctx.enter_context(tc.tile_pool(name="psm", bufs=8))
        # PSUM pools: 2 + 2 + 4 = 8 banks exactly.
        ps_small = ctx.enter_context(tc.tile_pool(name="ps_small", bufs=2,
                                                  space="PSUM"))
        ps_y = ctx.enter_context(tc.tile_pool(name="ps_y", bufs=2,
                                              space="PSUM"))
        ps_mm = ctx.enter_context(tc.tile_pool(name="ps_mm", bufs=4,
                                               space="PSUM"))
        pdram = ctx.enter_context(tc.tile_pool(name="pdram", bufs=2,
                                               space="DRAM"))

        def stage_front(t):
            """Load + stats + normalize + transpose (DMA/DVE/ACT only)."""
            row0 = t * TOK
            xt = px.tile([128, NCH, D], f32, tag="xt", bufs=3, name=f"xt{t}")
            for cch in range(NCH):
                nc.sync.dma_start(
                    out=xt[:, cch, :],
                    in_=x_d[row0 + cch * 128: row0 + (cch + 1) * 128, :])
            hb = px.tile([128, NCH, D], bf16, tag="hb", bufs=2, name=f"hb{t}")
            mv = psm.tile([128, NCH, 2], f32, tag="mv", name=f"mv{t}")
            for cch in range(NCH):
                st = psm.tile([128, 2, 6], f32, tag="st")
                xr = xt[:, cch, :].rearrange("p (s f) -> p s f", s=2)
                nc.vector.bn_stats(out=st[:, 0, :], in_=xr[:, 0, :])
                nc.vector.bn_stats(out=st[:, 1, :], in_=xr[:, 1, :])
                nc.vector.bn_aggr(out=mv[:, cch, :], in_=st)
            sd = psm.tile([128, NCH], f32, tag="sd")
            nc.scalar.activation(sd, mv[:, :, 1], AF.Sqrt, bias=ctiny,
                                 scale=float(D))
            ee = psm.tile([128, NCH], f32, tag="ee", name=f"ee{t}")
            nc.vector.reciprocal(ee, sd)
            bh = psm.tile([128, NCH], f32, tag="bh", name=f"bh{t}")
            nc.vector.scalar_tensor_tensor(
                out=bh, in0=mv[:, :, 0], scalar=-1.0, in1=ee,
                op0=OP.mult, op1=OP.mult)
            for cch in range(NCH):
                nc.scalar.activation(hb[:, cch, :], xt[:, cch, :], AF.Identity,
                                     bias=bh[:, cch:cch + 1],
                                     scale=ee[:, cch:cch + 1])
            hbT = px.tile([128, KD, TOK], bf16, tag="hbT", bufs=2,
                          name=f"hbT{t}")
            if interleaved_t:
                for cch in range(NCH):
                    nc.sync.dma_start_transpose(
                        out=hbT[:, :, cch * 128:(cch + 1) * 128],
                        in_=hb[:, cch, :])
            else:
                for cch in range(NCH):
                    for dch in range(KD):
                        nc.sync.dma_start_transpose(
                            out=hbT[:, dch, cch * 128:(cch + 1) * 128],
                            in_=hb[:, cch, dch * 128:(dch + 1) * 128])
            return xt, hbT

        def stage_back(t, xt, hbT):
            row0 = t * TOK
            # --- A0 = a_norm @ h  [16, TOK] -------------------------------
            a0p = ps_small.tile([A, TOK], f32, tag="small")
            for dch in range(KD):
                nc.tensor.matmul(a0p, lhsT=agt[:, dch, :], rhs=hbT[:, dch, :],
                                 start=(dch == 0), stop=(dch == KD - 1))
            a0 = psm.tile([48, TOK], bf16, tag="a0", bufs=2)
            nc.vector.tensor_copy(out=a0[0:A, :], in_=a0p)
            nc.sync.dma_start(out=a0[32:32 + A, :], in_=a0[0:A, :])

            # --- expand (2-way row-packed) + relu + square ----------------
            rbig = px.tile([128, KD, TOK], bf16, tag="rbig", bufs=2)
            ubig = px.tile([128, KD, TOK], bf16, tag="ubig", bufs=2)
            for kg in range(KD // 2):
                k0, k1 = 2 * kg, 2 * kg + 1
                up0 = ps_mm.tile([128, TOK], f32, tag="mmout")
                nc.tensor.matmul(up0, lhsT=w1e[0:A, k0, :], rhs=a0[0:A, :],
                                 start=True, stop=True)
                up1 = ps_mm.tile([128, TOK], f32, tag="mmout")
                nc.tensor.matmul(up1, lhsT=w1e[32:32 + A, k1, :],
                                 rhs=a0[32:32 + A, :], start=True, stop=True)
                nc.scalar.activation(rbig[:, k0, :], up0, AF.Relu,
                                     bias=biasu[:, k0:k0 + 1], scale=1.0)
                nc.scalar.activation(rbig[:, k1, :], up1, AF.Relu,
                                     bias=biasu[:, k1:k1 + 1], scale=1.0)
                nc.vector.tensor_mul(ubig[:, k0, :], rbig[:, k0, :],
                                     rbig[:, k0, :])
                nc.gpsimd.tensor_mul(ubig[:, k1, :], rbig[:, k1, :],
                                     rbig[:, k1, :])

            # --- comp matmul + centered bias + square ---------------------
            yb = px.tile([128, 4, TOK], bf16, tag="yb", bufs=2)
            sqy = px.tile([128, 4, TOK], bf16, tag="sqy", bufs=2)
            for j in range(4):
                yp = ps_y.tile([128, TOK], f32, tag="ypre")
                nc.tensor.matmul(yp[0:64, :], lhsT=w2c[:, 2 * j, :],
                                 rhs=ubig[:, 2 * j, :], start=True, stop=True)
                nc.tensor.matmul(yp[64:128, :], lhsT=w2c[:, 2 * j + 1, :],
                                 rhs=ubig[:, 2 * j + 1, :], start=True,
                                 stop=True, tile_position=(0, 64))
                nc.scalar.activation(yb[:, j, :], yp, AF.Identity,
                                     bias=b2f[:, j:j + 1], scale=1.0)
                nc.gpsimd.tensor_mul(sqy[:, j, :], yb[:, j, :], yb[:, j, :])

            # --- per-comp variance via PE, rstd = exp(-ln(var+eps)/2) -----
            vst = ps_small.tile([C, TOK], f32, tag="small")
            for j in range(4):
                nc.tensor.matmul(vst, lhsT=vstl[:, j, :], rhs=sqy[:, j, :],
                                 start=(j == 0), stop=(j == 3))
            sd2 = psm.tile([C, TOK], f32, tag="sd2", bufs=2)
            nc.scalar.activation(sd2, vst, AF.Ln, bias=ceps, scale=1.0)
            rrb = psm.tile([C, TOK], bf16, tag="rrb", bufs=2)
            nc.scalar.activation(rrb, sd2, AF.Exp, bias=czero, scale=-0.5)
            rrd = pdram.tile([C, TOK], bf16, tag="rrd")
            nc.sync.dma_start(out=rrd, in_=rrb)
            rbB = px.tile([128, 4, TOK], bf16, tag="rbB", bufs=2)
            import concourse.bass as bass_mod
            for p1 in range(2):
                src = bass_mod.AP(
                    tensor=rrd.tensor if hasattr(rrd, 'tensor') else rrd,
                    offset=(rrd.offset if hasattr(rrd, 'offset') else 0)
                    + p1 * TOK,
                    ap=[[0, 64], [2 * TOK, 4], [1, TOK]])
                nc.sync.dma_start(out=rbB[64 * p1:64 * (p1 + 1), :, :],
                                  in_=src)

            ycT = px.tile([128, 4, TOK], bf16, tag="ycT", bufs=2)
            for j in range(4):
                nc.vector.tensor_mul(ycT[:, j, :], yb[:, j, :], rbB[:, j, :])

            # --- proj (operand-swapped -> token-major) + residual ---------
            for cch in range(NCH):
                osb = px.tile([128, D], f32, tag="osb", bufs=3)
                for hf in range(2):
                    ud = ps_mm.tile([128, 512], f32, tag="mmout")
                    for j in range(4):
                        nc.tensor.matmul(
                            ud, lhsT=ycT[:, j, cch * 128:(cch + 1) * 128],
                            rhs=wpf[:, j, hf, :],
                            start=(j == 0),
                            stop=(j == 3 and not use_const))
                    if use_const:
                        nc.tensor.matmul(ud, lhsT=ones1, rhs=cvec[:, hf, :],
                                         start=False, stop=True)
                    nc.vector.tensor_add(
                        osb[:, hf * 512:(hf + 1) * 512], ud,
                        xt[:, cch, hf * 512:(hf + 1) * 512])
                nc.sync.dma_start(
                    out=out_d[row0 + cch * 128: row0 + (cch + 1) * 128, :],
                    in_=osb[:, :])

        pend = None
        for t in range(ntile + 1):
            if t < ntile:
                fr = stage_front(t)
            if pend is not None:
                stage_back(t - 1, *pend)
            pend = fr if t < ntile else None

    nc.compile()
    return nc


def _pack_params(anchors, ln_g, W1, b1, W2, b2, cg, cb, Wp, bp, gate,
                 interleaved_t=True):
    f32 = np.float32
    anchors = anchors.astype(f32)
    an = anchors / np.maximum(
        np.linalg.norm(anchors.astype(np.float64), axis=1, keepdims=True),
        1e-12).astype(f32)
    ag = (an * ln_g[None, :].astype(f32)).astype(f32)  # [A, D]

    # agt[p, s, m] = ag[m, d(p,s)]
    agt = np.zeros((128, KD, A), f32)
    dd = np.arange(D)
    if interleaved_t:
        pidx, sidx = dd // KD, dd % KD
    else:
        pidx, sidx = dd % 128, dd // 128
    agt[pidx, sidx, :] = ag.T[dd, :]

    # W1exp[m, f] with m=j*C+k2, f=k*128+e -> value W1[k, j, e] iff k2==k
    W1 = W1.astype(f32)
    w1exp = np.zeros((A, C, E2), f32)
    for m in range(A):
        j, k2 = m // C, m % C
        w1exp[m, k2, :] = W1[k2, j, :]
    w1e = (-w1exp).reshape(A, C, E2)  # [16, 8, 128] (f = k*128+e)
    sf = w1exp.sum(axis=0)  # [C, E2]
    biasu = (sf + b1.astype(f32)).T.copy()  # [128, C] (partition=e, col=k)

    W2 = W2.astype(f32)
    w2m = W2.mean(axis=2, keepdims=True)
    w2cent = W2 - w2m  # [C, E2, DC]
    w2c = np.transpose(w2cent, (1, 0, 2)).copy()  # [128, C, 64]
    b2c = b2.astype(f32) - b2.astype(f32).mean(axis=1, keepdims=True)  # [C, DC]

    b2f = np.zeros((128, 4), f32)
    vstl = np.zeros((128, 4, C), f32)
    for j in range(4):
        for p in range(128):
            kk = 2 * j + p // 64
            b2f[p, j] = b2c[kk, p % 64]
            vstl[p, j, kk] = 1.0 / DC

    sig = (1.0 / (1.0 + np.exp(-gate.astype(np.float64)))).astype(f32)  # [D]
    wpfold = (cg.astype(f32).reshape(C * DC, 1) * Wp.astype(f32)) * sig[None, :]
    wpf = np.ascontiguousarray(
        wpfold.reshape(4, 128, 2, 512).transpose(1, 0, 2, 3))

    const = (cb.astype(f32).reshape(-1) @ Wp.astype(f32) + bp.astype(f32)) * sig
    use_const = bool(np.max(np.abs(const)) > 0)

    import ml_dtypes
    bf16 = ml_dtypes.bfloat16
    params = dict(
        agt=agt.astype(bf16),
        w1e=w1e.astype(bf16),
        biasu=biasu.astype(f32),
        w2c=w2c.astype(bf16),
        vstl=vstl.astype(bf16),
        b2f=b2f.astype(f32),
        wpf=wpf.astype(bf16),
    )
    if use_const:
        params["cvec"] = const.reshape(1, 2, 512).astype(bf16)
    return params, use_const


def kernel(**inputs):
    x = np.asarray(inputs["x"], dtype=np.float32)
    ln_g = np.asarray(inputs["ln_g"], dtype=np.float32)
    ln_b = np.asarray(inputs["ln_b"], dtype=np.float32)

    fast = (np.allclose(ln_g, 1.0, atol=1e-12) and
            np.allclose(ln_b, 0.0, atol=1e-12))
    if not fast:
        return _np_reference(
            x, *[np.asarray(inputs[k], dtype=np.float32) for k in
                 ("anchors", "ln_g", "ln_b", "W1", "b1", "W2", "b2", "cg",
                  "cb", "Wp", "bp", "gate")])

    params, use_const = _pack_params(
        inputs["anchors"], ln_g, inputs["W1"], inputs["b1"], inputs["W2"],
        inputs["b2"], inputs["cg"], inputs["cb"], inputs["Wp"], inputs["bp"],
        inputs["gate"], interleaved_t=INTERLEAVED_T)

    nc = _build_program(S, use_const, INTERLEAVED_T, USE_RECIP_APPROX)

    from concourse.bass_utils import run_bass_kernel_spmd
    in_maps = []
    for b in range(NCORES):
        m = dict(params)
        m["x"] = np.ascontiguousarray(x[b])
        in_maps.append(m)
    res = run_bass_kernel_spmd(nc, in_maps, core_ids=list(range(NCORES)))
    out = np.stack([res.results[b]["out"] for b in range(NCORES)], axis=0)
    return out.reshape(B, S, D).astype(np.float32)


INTERLEAVED_T = True
USE_RECIP_APPROX = True


# revision 14
# speedup vs baseline: 1.2842x; 1.0786x over previous
"""Trainium2 Bass kernel for nn_ConstellationRelay.

Computation (per token, D=1024, A=16 anchors, C=8 comps, dc=64):
  h   = l2norm(layernorm(x; ln_g, ln_b))
  tri = 1 - h @ l2norm(anchors).T                       (N, 16)
  u   = relu(einsum('nak,kae->nke', tri_g, W1) + b1)^2  (N, 8, 128)
  y   = layernorm_c(u @ W2 + b2; cg, cb)                (N, 8, 64)
  out = x + sigmoid(gate) * (y.flat @ Wp + bp)

Strategy: pure data-parallel over batch (one of 8 NeuronCores per batch row).
On-device fast path requires ln_g==1, ln_b==0 (always true for this problem's
setup_inputs); every other parameter is handled generally via host-side
folding:
  * h = (x - mu)/sqrt(1024*var)  -- eps cancels exactly through the l2norm
  * tri/W1 stage folded into two small matmuls (A0 = a_norm @ h; expand with
    -W1exp and bias (sum_m W1exp + b1) applied in the ReLU activation)
  * comp-LN mean-subtraction folded into centered W2/b2 (host)
  * cg, cb, bp, sigmoid(gate) folded into Wp/const (host)
Layout: token-major for stats/residual, feature-major (via DMA-transpose of
bf16 h) for all matmuls; proj matmul operand-swapped so the residual add
lands token-major in PSUM.
"""

import functools
import os
import sys

import numpy as np

for _p in ("/opt/trn_rl_repo",):
    if _p not in sys.path and os.path.isdir(_p):
        sys.path.insert(0, _p)

B, S, D = 8, 4096, 1024
A, C, DC = 16, 8, 64
APC = A // C  # anchors per compartment
E2 = 2 * DC  # 128, expanded width per comp
NCORES = 8
TOK = 512  # tokens per pipeline tile
NTILE = S // TOK  # 8
NCH = TOK // 128  # 4 token chunks of 128 per tile
KD = D // 128  # 8 feature chunks


def _np_reference(x, anchors, ln_g, ln_b, W1, b1, W2, b2, cg, cb, Wp, bp, gate):
    """Pure-numpy fallback, mirrors reference.py (used only if ln_g/ln_b
    deviate from the values this problem's setup_inputs produces)."""
    x = x.astype(np.float32)
    N = x.shape[0] * x.shape[1]
    xf = x.reshape(N, D)
    mu = xf.mean(-1, keepdims=True)
    var = ((xf - mu) ** 2).mean(-1, keepdims=True)
    h = (xf - mu) / np.sqrt(var + 1e-5) * ln_g + ln_b
    h = h / np.maximum(np.linalg.norm(h, axis=-1, keepdims=True), 1e-12)
    a = anchors / np.maximum(np.linalg.norm(anchors, axis=-1, keepdims=True), 1e-12)
    tri = 1.0 - h @ a.T
    g = tri.reshape(N, APC, C)
    u = np.einsum("nak,kae->nke", g, W1) + b1
    u = np.square(np.maximum(u, 0.0))
    y = np.einsum("nke,ked->nkd", u, W2) + b2
    muy = y.mean(-1, keepdims=True)
    vy = ((y - muy) ** 2).mean(-1, keepdims=True)
    y = (y - muy) / np.sqrt(vy + 1e-5) * cg + cb
    upd = y.reshape(N, C * DC) @ Wp + bp
    sig = 1.0 / (1.0 + np.exp(-gate))
    return (xf + sig * upd).reshape(x.shape).astype(np.float32)


@functools.lru_cache(maxsize=4)
def _build_program(n_tokens=S, use_const=False, interleaved_t=True,
                   use_recip_approx=True):
    """Build + schedule the single-core Bass program (same program runs SPMD
    on all 8 cores).

    interleaved_t: if True, the 3D-output dma_start_transpose writes feature
    d of h to (partition=d//KD, sub=d%KD); host packs the anchor matrix to
    match.  If False, use 32 plain 128x128 transposes with the natural
    d=(chunk*128+p) layout.
    """
    import concourse.bacc as bacc
    import concourse.mybir as mybir
    import concourse.tile as tile

    f32 = mybir.dt.float32
    bf16 = mybir.dt.bfloat16
    AF = mybir.ActivationFunctionType
    OP = mybir.AluOpType

    ntile = n_tokens // TOK

    nc = bacc.Bacc("TRN2", target_bir_lowering=False, debug=False,
                   num_devices=NCORES)

    x_d = nc.dram_tensor("x", [n_tokens, D], f32, kind="ExternalInput")
    agt_d = nc.dram_tensor("agt", [128, KD, A], bf16, kind="ExternalInput")
    w1e_d = nc.dram_tensor("w1e", [A, KD, 128], bf16, kind="ExternalInput")
    biasu_d = nc.dram_tensor("biasu", [128, KD], f32, kind="ExternalInput")
    w2c_d = nc.dram_tensor("w2c", [128, C, DC], bf16, kind="ExternalInput")
    vstl_d = nc.dram_tensor("vstl", [128, 4, C], bf16, kind="ExternalInput")
    b2f_d = nc.dram_tensor("b2f", [128, 4], f32, kind="ExternalInput")
    wpf_d = nc.dram_tensor("wpf", [128, 4, 2, 512], bf16, kind="ExternalInput")
    sel_d = nc.dram_tensor("sel", [C, C, 64], bf16, kind="ExternalInput")
    cvec_d = nc.dram_tensor("cvec", [1, 2, 512], bf16, kind="ExternalInput") \
        if use_const else None
    out_d = nc.dram_tensor("out", [n_tokens, D], f32, kind="ExternalOutput")

    from contextlib import ExitStack

    with tile.TileContext(nc) as tc, ExitStack() as ctx:
        pp = ctx.enter_context(tc.tile_pool(name="params", bufs=1))
        agt = pp.tile([128, KD, A], bf16)
        nc.sync.dma_start(out=agt, in_=agt_d[:, :, :])
        w1e = pp.tile([48, KD, 128], bf16)
        nc.sync.dma_start(out=w1e[0:A, :, :], in_=w1e_d[:, :, :])
        nc.sync.dma_start(out=w1e[32:32 + A, :, :], in_=w1e_d[:, :, :])
        biasu = pp.tile([128, KD], f32)
        nc.sync.dma_start(out=biasu, in_=biasu_d[:, :])
        w2c = pp.tile([128, C, DC], bf16)
        nc.sync.dma_start(out=w2c, in_=w2c_d[:, :, :])
        vstl = pp.tile([128, 4, C], bf16)
        nc.sync.dma_start(out=vstl, in_=vstl_d[:, :, :])
        b2f = pp.tile([128, 4], f32)
        nc.sync.dma_start(out=b2f, in_=b2f_d[:, :])
        wpf = pp.tile([128, 4, 2, 512], bf16)
        nc.sync.dma_start(out=wpf, in_=wpf_d[:, :, :, :])
        sel = pp.tile([C, C, 64], bf16)
        nc.sync.dma_start(out=sel, in_=sel_d[:, :, :])
        if use_const:
            cvec = pp.tile([1, 2, 512], bf16)
            nc.sync.dma_start(out=cvec, in_=cvec_d[:, :, :])
            ones1 = pp.tile([1, 128], bf16)
            nc.vector.memset(ones1, 1.0)
        ctiny = pp.tile([128, 1], f32)
        nc.vector.memset(ctiny, 1e-38)
        ceps = pp.tile([C, 1], f32)
        nc.vector.memset(ceps, 1e-5)
        czero = pp.tile([C, 1], f32)
        nc.vector.memset(czero, 0.0)

        px = ctx.enter_context(tc.tile_pool(name="px", bufs=2))
        psm = ctx.enter_context(tc.tile_pool(name="psm", bufs=8))
        # PSUM pools: 2 + 2 + 4 = 8 banks exactly.
        ps_small = ctx.enter_context(tc.tile_pool(name="ps_small", bufs=2,
                                                  space="PSUM"))
        ps_y = ctx.enter_context(tc.tile_pool(name="ps_y", bufs=2,
                                              space="PSUM"))
        ps_mm = ctx.enter_context(tc.tile_pool(name="ps_mm", bufs=4,
                                               space="PSUM"))

        def stage_front(t):
            """Load + stats + normalize + transpose (DMA/DVE/ACT only)."""
            row0 = t * TOK
            xt = px.tile([128, NCH, D], f32, tag="xt", bufs=3, name=f"xt{t}")
            for cch in range(NCH):
                nc.sync.dma_start(
                    out=xt[:, cch, :],
                    in_=x_d[row0 + cch * 128: row0 + (cch + 1) * 128, :])
            hb = px.tile([128, NCH, D], bf16, tag="hb", bufs=2, name=f"hb{t}")
            mv = psm.tile([128, NCH, 2], f32, tag="mv", name=f"mv{t}")
            for cch in range(NCH):
                st = psm.tile([128, 2, 6], f32, tag="st")
                xr = xt[:, cch, :].rearrange("p (s f) -> p s f", s=2)
                nc.vector.bn_stats(out=st[:, 0, :], in_=xr[:, 0, :])
                nc.vector.bn_stats(out=st[:, 1, :], in_=xr[:, 1, :])
                nc.vector.bn_aggr(out=mv[:, cch, :], in_=st)
            sd = psm.tile([128, NCH], f32, tag="sd")
            nc.scalar.activation(sd, mv[:, :, 1], AF.Sqrt, bias=ctiny,
                                 scale=float(D))
            ee = psm.tile([128, NCH], f32, tag="ee", name=f"ee{t}")
            nc.vector.reciprocal(ee, sd)
            bh = psm.tile([128, NCH], f32, tag="bh", name=f"bh{t}")
            nc.vector.scalar_tensor_tensor(
                out=bh, in0=mv[:, :, 0], scalar=-1.0, in1=ee,
                op0=OP.mult, op1=OP.mult)
            for cch in range(NCH):
                nc.scalar.activation(hb[:, cch, :], xt[:, cch, :], AF.Identity,
                                     bias=bh[:, cch:cch + 1],
                                     scale=ee[:, cch:cch + 1])
            hbT = px.tile([128, KD, TOK], bf16, tag="hbT", bufs=2,
                          name=f"hbT{t}")
            if interleaved_t:
                for cch in range(NCH):
                    nc.sync.dma_start_transpose(
                        out=hbT[:, :, cch * 128:(cch + 1) * 128],
                        in_=hb[:, cch, :])
            else:
                for cch in range(NCH):
                    for dch in range(KD):
                        nc.sync.dma_start_transpose(
                            out=hbT[:, dch, cch * 128:(cch + 1) * 128],
                            in_=hb[:, cch, dch * 128:(dch + 1) * 128])
            return xt, hbT

        def stage_mid(t, xt, hbT):
            # --- A0 = a_norm @ h  [16, TOK] -------------------------------
            a0p = ps_small.tile([A, TOK], f32, tag="small")
            for dch in range(KD):
                nc.tensor.matmul(a0p, lhsT=agt[:, dch, :], rhs=hbT[:, dch, :],
                                 start=(dch == 0), stop=(dch == KD - 1))
            a0 = psm.tile([48, TOK], bf16, tag="a0", bufs=2)
            nc.vector.tensor_copy(out=a0[0:A, :], in_=a0p)
            nc.sync.dma_start(out=a0[32:32 + A, :], in_=a0[0:A, :])

            # --- expand (2-way row-packed) + relu + square ----------------
            rbig = px.tile([128, KD, TOK], bf16, tag="rbig", bufs=2)
            ubig = px.tile([128, KD, TOK], bf16, tag="ubig", bufs=2)
            for kg in range(KD // 2):
                k0, k1 = 2 * kg, 2 * kg + 1
                up0 = ps_mm.tile([128, TOK], f32, tag="mmout")
                nc.tensor.matmul(up0, lhsT=w1e[0:A, k0, :], rhs=a0[0:A, :],
                                 start=True, stop=True)
                up1 = ps_mm.tile([128, TOK], f32, tag="mmout")
                nc.tensor.matmul(up1, lhsT=w1e[32:32 + A, k1, :],
                                 rhs=a0[32:32 + A, :], start=True, stop=True)
                nc.scalar.activation(rbig[:, k0, :], up0, AF.Relu,
                                     bias=biasu[:, k0:k0 + 1], scale=1.0)
                nc.scalar.activation(rbig[:, k1, :], up1, AF.Relu,
                                     bias=biasu[:, k1:k1 + 1], scale=1.0)
                nc.vector.tensor_mul(ubig[:, k0, :], rbig[:, k0, :],
                                     rbig[:, k0, :])
                nc.gpsimd.tensor_mul(ubig[:, k1, :], rbig[:, k1, :],
                                     rbig[:, k1, :])

            # --- comp matmul + centered bias + square ---------------------
            yb = px.tile([128, 4, TOK], bf16, tag="yb", bufs=3,
                         name=f"yb{t}")
            sqy = px.tile([128, 4, TOK], bf16, tag="sqy", bufs=2)
            for j in range(4):
                yp = ps_y.tile([128, TOK], f32, tag="ypre")
                nc.tensor.matmul(yp[0:64, :], lhsT=w2c[:, 2 * j, :],
                                 rhs=ubig[:, 2 * j, :], start=True, stop=True)
                nc.tensor.matmul(yp[64:128, :], lhsT=w2c[:, 2 * j + 1, :],
                                 rhs=ubig[:, 2 * j + 1, :], start=True,
                                 stop=True, tile_position=(0, 64))
                nc.scalar.activation(yb[:, j, :], yp, AF.Identity,
                                     bias=b2f[:, j:j + 1], scale=1.0)
                nc.gpsimd.tensor_mul(sqy[:, j, :], yb[:, j, :], yb[:, j, :])

            # --- per-comp variance via PE; rstd = 1/sqrt(var+eps) ---------
            vst = ps_small.tile([C, TOK], f32, tag="small")
            for j in range(4):
                nc.tensor.matmul(vst, lhsT=vstl[:, j, :], rhs=sqy[:, j, :],
                                 start=(j == 0), stop=(j == 3))
            sd2 = psm.tile([C, TOK], f32, tag="sd2", bufs=2)
            nc.scalar.activation(sd2, vst, AF.Sqrt, bias=ceps, scale=1.0)
            rr = psm.tile([C, TOK], f32, tag="rr", bufs=2)
            if use_recip_approx:
                nc.vector.reciprocal_approx_fast(out=rr, in_=sd2)
            else:
                nc.vector.reciprocal(out=rr, in_=sd2)
            rrb = psm.tile([C, TOK], bf16, tag="rrb", bufs=3, name=f"rrb{t}")
            nc.vector.tensor_copy(out=rrb, in_=rr)
            return xt, yb, rrb

        def stage_back(t, xt, yb, rrb):
            row0 = t * TOK
            # rstd broadcast via selector matmuls; ycT = yb * rstd
            ycT = px.tile([128, 4, TOK], bf16, tag="ycT", bufs=2)
            for j in range(4):
                rbP = ps_mm.tile([128, TOK], f32, tag="mmout")
                nc.tensor.matmul(rbP[0:64, :], lhsT=sel[:, 2 * j, :],
                                 rhs=rrb, start=True, stop=True)
                nc.tensor.matmul(rbP[64:128, :], lhsT=sel[:, 2 * j + 1, :],
                                 rhs=rrb, start=True, stop=True,
                                 tile_position=(0, 64))
                nc.vector.tensor_mul(ycT[:, j, :], yb[:, j, :], rbP)

            # --- proj (operand-swapped -> token-major) + residual ---------
            for cch in range(NCH):
                osb = px.tile([128, D], f32, tag="osb", bufs=3)
                for hf in range(2):
                    ud = ps_mm.tile([128, 512], f32, tag="mmout")
                    for j in range(4):
                        nc.tensor.matmul(
                            ud, lhsT=ycT[:, j, cch * 128:(cch + 1) * 128],
                            rhs=wpf[:, j, hf, :],
                            start=(j == 0),
                            stop=(j == 3 and not use_const))
                    if use_const:
                        nc.tensor.matmul(ud, lhsT=ones1, rhs=cvec[:, hf, :],
                                         start=False, stop=True)
                    nc.vector.tensor_add(
                        osb[:, hf * 512:(hf + 1) * 512], ud,
                        xt[:, cch, hf * 512:(hf + 1) * 512])
                nc.sync.dma_start(
                    out=out_d[row0 + cch * 128: row0 + (cch + 1) * 128, :],
                    in_=osb[:, :])

        fr = {}
        md = {}
        for t in range(ntile + 2):
            if t < ntile:
                fr[t] = stage_front(t)
            if 1 <= t <= ntile:
                md[t - 1] = stage_mid(t - 1, *fr.pop(t - 1))
            if t >= 2:
                stage_back(t - 2, *md.pop(t - 2))

    nc.compile()
    return nc


def _pack_params(anchors, ln_g, W1, b1, W2, b2, cg, cb, Wp, bp, gate,
                 interleaved_t=True):
    f32 = np.float32
    anchors = anchors.astype(f32)
    an = anchors / np.maximum(
        np.linalg.norm(anchors.astype(np.float64), axis=1, keepdims=True),
        1e-12).astype(f32)
    ag = (an * ln_g[None, :].astype(f32)).astype(f32)  # [A, D]

    # agt[p, s, m] = ag[m, d(p,s)]
    agt = np.zeros((128, KD, A), f32)
    dd = np.arange(D)
    if interleaved_t:
        pidx, sidx = dd // KD, dd % KD
    else:
        pidx, sidx = dd % 128, dd // 128
    agt[pidx, sidx, :] = ag.T[dd, :]

    # W1exp[m, f] with m=j*C+k2, f=k*128+e -> value W1[k, j, e] iff k2==k
    W1 = W1.astype(f32)
    w1exp = np.zeros((A, C, E2), f32)
    for m in range(A):
        j, k2 = m // C, m % C
        w1exp[m, k2, :] = W1[k2, j, :]
    w1e = (-w1exp).reshape(A, C, E2)  # [16, 8, 128] (f = k*128+e)
    sf = w1exp.sum(axis=0)  # [C, E2]
    biasu = (sf + b1.astype(f32)).T.copy()  # [128, C] (partition=e, col=k)

    W2 = W2.astype(f32)
    w2m = W2.mean(axis=2, keepdims=True)
    w2cent = W2 - w2m  # [C, E2, DC]
    w2c = np.transpose(w2cent, (1, 0, 2)).copy()  # [128, C, 64]
    b2c = b2.astype(f32) - b2.astype(f32).mean(axis=1, keepdims=True)  # [C, DC]

    b2f = np.zeros((128, 4), f32)
    vstl = np.zeros((128, 4, C), f32)
    for j in range(4):
        for p in range(128):
            kk = 2 * j + p // 64
            b2f[p, j] = b2c[kk, p % 64]
            vstl[p, j, kk] = 1.0 / DC

    sig = (1.0 / (1.0 + np.exp(-gate.astype(np.float64)))).astype(f32)  # [D]
    wpfold = (cg.astype(f32).reshape(C * DC, 1) * Wp.astype(f32)) * sig[None, :]
    wpf = np.ascontiguousarray(
        wpfold.reshape(4, 128, 2, 512).transpose(1, 0, 2, 3))

    const = (cb.astype(f32).reshape(-1) @ Wp.astype(f32) + bp.astype(f32)) * sig
    use_const = bool(np.max(np.abs(const)) > 0)

    import ml_dtypes
    bf16 = ml_dtypes.bfloat16
    sel = np.zeros((C, C, 64), f32)
    for r in range(C):
        sel[r, r, :] = 1.0

    params = dict(
        sel=sel.astype(bf16),
        agt=agt.astype(bf16),
        w1e=w1e.astype(bf16),
        biasu=biasu.astype(f32),
        w2c=w2c.astype(bf16),
        vstl=vstl.astype(bf16),
        b2f=b2f.astype(f32),
        wpf=wpf.astype(bf16),
    )
    if use_const:
        params["cvec"] = const.reshape(1, 2, 512).astype(bf16)
    return params, use_const


def kernel(**inputs):
    x = np.asarray(inputs["x"], dtype=np.float32)
    ln_g = np.asarray(inputs["ln_g"], dtype=np.float32)
    ln_b = np.asarray(inputs["ln_b"], dtype=np.float32)

    fast = (np.allclose(ln_g, 1.0, atol=1e-12) and
            np.allclose(ln_b, 0.0, atol=1e-12))
    if not fast:
        return _np_reference(
            x, *[np.asarray(inputs[k], dtype=np.float32) for k in
                 ("anchors", "ln_g", "ln_b", "W1", "b1", "W2", "b2", "cg",
                  "cb", "Wp", "bp", "gate")])

    params, use_const = _pack_params(
        inputs["anchors"], ln_g, inputs["W1"], inputs["b1"], inputs["W2"],
        inputs["b2"], inputs["cg"], inputs["cb"], inputs["Wp"], inputs["bp"],
        inputs["gate"], interleaved_t=INTERLEAVED_T)

    nc = _build_program(S, use_const, INTERLEAVED_T, USE_RECIP_APPROX)

    from concourse.bass_utils import run_bass_kernel_spmd
    in_maps = []
    for b in range(NCORES):
        m = dict(params)
        m["x"] = np.ascontiguousarray(x[b])
        in_maps.append(m)
    res = run_bass_kernel_spmd(nc, in_maps, core_ids=list(range(NCORES)))
    out = np.stack([res.results[b]["out"] for b in range(NCORES)], axis=0)
    return out.reshape(B, S, D).astype(np.float32)


INTERLEAVED_T = True
USE_RECIP_APPROX = True


# revision 15
# speedup vs baseline: 1.4004x; 1.0905x over previous
"""Trainium2 Bass kernel for nn_ConstellationRelay.

Computation (per token, D=1024, A=16 anchors, C=8 comps, dc=64):
  h   = l2norm(layernorm(x; ln_g, ln_b))
  tri = 1 - h @ l2norm(anchors).T                       (N, 16)
  u   = relu(einsum('nak,kae->nke', tri_g, W1) + b1)^2  (N, 8, 128)
  y   = layernorm_c(u @ W2 + b2; cg, cb)                (N, 8, 64)
  out = x + sigmoid(gate) * (y.flat @ Wp + bp)

Strategy: pure data-parallel over batch (one of 8 NeuronCores per batch row).
On-device fast path requires ln_g==1, ln_b==0 (always true for this problem's
setup_inputs); every other parameter is handled generally via host-side
folding:
  * h = (x - mu)/sqrt(1024*var)  -- eps cancels exactly through the l2norm
  * tri/W1 stage folded into two small matmuls (A0 = a_norm @ h; expand with
    -W1exp and bias (sum_m W1exp + b1) applied in the ReLU activation)
  * comp-LN mean-subtraction folded into centered W2/b2 (host)
  * cg, cb, bp, sigmoid(gate) folded into Wp/const (host)
Layout: token-major for stats/residual, feature-major (via DMA-transpose of
bf16 h) for all matmuls; proj matmul operand-swapped so the residual add
lands token-major in PSUM.
"""

import functools
import os
import sys

import numpy as np

for _p in ("/opt/trn_rl_repo",):
    if _p not in sys.path and os.path.isdir(_p):
        sys.path.insert(0, _p)

B, S, D = 8, 4096, 1024
A, C, DC = 16, 8, 64
APC = A // C  # anchors per compartment
E2 = 2 * DC  # 128, expanded width per comp
NCORES = 8
TOK = 512  # tokens per pipeline tile
NTILE = S // TOK  # 8
NCH = TOK // 128  # 4 token chunks of 128 per tile
KD = D // 128  # 8 feature chunks


def _np_reference(x, anchors, ln_g, ln_b, W1, b1, W2, b2, cg, cb, Wp, bp, gate):
    """Pure-numpy fallback, mirrors reference.py (used only if ln_g/ln_b
    deviate from the values this problem's setup_inputs produces)."""
    x = x.astype(np.float32)
    N = x.shape[0] * x.shape[1]
    xf = x.reshape(N, D)
    mu = xf.mean(-1, keepdims=True)
    var = ((xf - mu) ** 2).mean(-1, keepdims=True)
    h = (xf - mu) / np.sqrt(var + 1e-5) * ln_g + ln_b
    h = h / np.maximum(np.linalg.norm(h, axis=-1, keepdims=True), 1e-12)
    a = anchors / np.maximum(np.linalg.norm(anchors, axis=-1, keepdims=True), 1e-12)
    tri = 1.0 - h @ a.T
    g = tri.reshape(N, APC, C)
    u = np.einsum("nak,kae->nke", g, W1) + b1
    u = np.square(np.maximum(u, 0.0))
    y = np.einsum("nke,ked->nkd", u, W2) + b2
    muy = y.mean(-1, keepdims=True)
    vy = ((y - muy) ** 2).mean(-1, keepdims=True)
    y = (y - muy) / np.sqrt(vy + 1e-5) * cg + cb
    upd = y.reshape(N, C * DC) @ Wp + bp
    sig = 1.0 / (1.0 + np.exp(-gate))
    return (xf + sig * upd).reshape(x.shape).astype(np.float32)


@functools.lru_cache(maxsize=4)
def _build_program(n_tokens=S, use_const=False, interleaved_t=True,
                   use_recip_approx=True):
    """Build + schedule the single-core Bass program (same program runs SPMD
    on all 8 cores).

    interleaved_t: if True, the 3D-output dma_start_transpose writes feature
    d of h to (partition=d//KD, sub=d%KD); host packs the anchor matrix to
    match.  If False, use 32 plain 128x128 transposes with the natural
    d=(chunk*128+p) layout.
    """
    import concourse.bacc as bacc
    import concourse.mybir as mybir
    import concourse.tile as tile

    f32 = mybir.dt.float32
    bf16 = mybir.dt.bfloat16
    AF = mybir.ActivationFunctionType
    OP = mybir.AluOpType

    ntile = n_tokens // TOK

    nc = bacc.Bacc("TRN2", target_bir_lowering=False, debug=False,
                   num_devices=NCORES)

    x_d = nc.dram_tensor("x", [n_tokens, D], f32, kind="ExternalInput")
    agt_d = nc.dram_tensor("agt", [128, KD, A], bf16, kind="ExternalInput")
    w1e_d = nc.dram_tensor("w1e", [A, KD, 128], bf16, kind="ExternalInput")
    biasu_d = nc.dram_tensor("biasu", [128, KD], f32, kind="ExternalInput")
    w2c_d = nc.dram_tensor("w2c", [128, C, DC], bf16, kind="ExternalInput")
    vstl_d = nc.dram_tensor("vstl", [128, 4, C], bf16, kind="ExternalInput")
    b2f_d = nc.dram_tensor("b2f", [128, 4], f32, kind="ExternalInput")
    wpf_d = nc.dram_tensor("wpf", [128, 4, 2, 512], bf16, kind="ExternalInput")
    sel_d = nc.dram_tensor("sel", [C, C, 64], bf16, kind="ExternalInput")
    cvec_d = nc.dram_tensor("cvec", [1, 2, 512], bf16, kind="ExternalInput") \
        if use_const else None
    out_d = nc.dram_tensor("out", [n_tokens, D], f32, kind="ExternalOutput")

    from contextlib import ExitStack

    with tile.TileContext(nc) as tc, ExitStack() as ctx:
        pp = ctx.enter_context(tc.tile_pool(name="params", bufs=1))
        agt = pp.tile([128, KD, A], bf16)
        nc.sync.dma_start(out=agt, in_=agt_d[:, :, :])
        w1e = pp.tile([A, KD, 128], bf16)
        nc.sync.dma_start(out=w1e, in_=w1e_d[:, :, :])
        biasu = pp.tile([128, KD], f32)
        nc.sync.dma_start(out=biasu, in_=biasu_d[:, :])
        w2c = pp.tile([128, C, DC], bf16)
        nc.sync.dma_start(out=w2c, in_=w2c_d[:, :, :])
        vstl = pp.tile([128, 4, C], bf16)
        nc.sync.dma_start(out=vstl, in_=vstl_d[:, :, :])
        b2f = pp.tile([128, 4], f32)
        nc.sync.dma_start(out=b2f, in_=b2f_d[:, :])
        wpf = pp.tile([128, 4, 2, 512], bf16)
        nc.sync.dma_start(out=wpf, in_=wpf_d[:, :, :, :])
        sel = pp.tile([C, C, 64], bf16)
        nc.sync.dma_start(out=sel, in_=sel_d[:, :, :])
        if use_const:
            cvec = pp.tile([1, 2, 512], bf16)
            nc.sync.dma_start(out=cvec, in_=cvec_d[:, :, :])
            ones1 = pp.tile([1, 128], bf16)
            nc.vector.memset(ones1, 1.0)
        ctiny = pp.tile([128, 1], f32)
        nc.vector.memset(ctiny, 1e-38)
        ceps = pp.tile([C, 1], f32)
        nc.vector.memset(ceps, 1e-5)
        czero = pp.tile([C, 1], f32)
        nc.vector.memset(czero, 0.0)

        px = ctx.enter_context(tc.tile_pool(name="px", bufs=2))
        psm = ctx.enter_context(tc.tile_pool(name="psm", bufs=8))
        # PSUM pools: 2 + 2 + 4 = 8 banks exactly.
        ps_small = ctx.enter_context(tc.tile_pool(name="ps_small", bufs=2,
                                                  space="PSUM"))
        ps_y = ctx.enter_context(tc.tile_pool(name="ps_y", bufs=2,
                                              space="PSUM"))
        ps_mm = ctx.enter_context(tc.tile_pool(name="ps_mm", bufs=4,
                                               space="PSUM"))

        def stage_front(t):
            """Load + stats + normalize + transpose (DMA/DVE/ACT only)."""
            row0 = t * TOK
            xt = px.tile([128, NCH, D], f32, tag="xt", bufs=3, name=f"xt{t}")
            for cch in range(NCH):
                nc.sync.dma_start(
                    out=xt[:, cch, :],
                    in_=x_d[row0 + cch * 128: row0 + (cch + 1) * 128, :])
            hb = px.tile([128, NCH, D], bf16, tag="hb", bufs=2, name=f"hb{t}")
            mv = psm.tile([128, NCH, 2], f32, tag="mv", name=f"mv{t}")
            for cch in range(NCH):
                st = psm.tile([128, 2, 6], f32, tag="st")
                xr = xt[:, cch, :].rearrange("p (s f) -> p s f", s=2)
                nc.vector.bn_stats(out=st[:, 0, :], in_=xr[:, 0, :])
                nc.vector.bn_stats(out=st[:, 1, :], in_=xr[:, 1, :])
                nc.vector.bn_aggr(out=mv[:, cch, :], in_=st)
            sd = psm.tile([128, NCH], f32, tag="sd")
            nc.scalar.activation(sd, mv[:, :, 1], AF.Sqrt, bias=ctiny,
                                 scale=float(D))
            ee = psm.tile([128, NCH], f32, tag="ee", name=f"ee{t}")
            nc.vector.reciprocal(ee, sd)
            bh = psm.tile([128, NCH], f32, tag="bh", name=f"bh{t}")
            nc.vector.scalar_tensor_tensor(
                out=bh, in0=mv[:, :, 0], scalar=-1.0, in1=ee,
                op0=OP.mult, op1=OP.mult)
            for cch in range(NCH):
                nc.scalar.activation(hb[:, cch, :], xt[:, cch, :], AF.Identity,
                                     bias=bh[:, cch:cch + 1],
                                     scale=ee[:, cch:cch + 1])
            hbT = px.tile([128, KD, TOK], bf16, tag="hbT", bufs=2,
                          name=f"hbT{t}")
            if interleaved_t:
                for cch in range(NCH):
                    nc.sync.dma_start_transpose(
                        out=hbT[:, :, cch * 128:(cch + 1) * 128],
                        in_=hb[:, cch, :])
            else:
                for cch in range(NCH):
                    for dch in range(KD):
                        nc.sync.dma_start_transpose(
                            out=hbT[:, dch, cch * 128:(cch + 1) * 128],
                            in_=hb[:, cch, dch * 128:(dch + 1) * 128])
            return xt, hbT

        def stage_mid_a0(t, xt, hbT):
            # --- A0 = a_norm @ h  [16, TOK] -------------------------------
            a0p = ps_small.tile([A, TOK], f32, tag="small")
            for dch in range(KD):
                nc.tensor.matmul(a0p, lhsT=agt[:, dch, :], rhs=hbT[:, dch, :],
                                 start=(dch == 0), stop=(dch == KD - 1))
            a0 = psm.tile([A, TOK], bf16, tag="a0", bufs=2)
            nc.scalar.copy(out=a0, in_=a0p)
            return a0

        def stage_mid(t, xt, hbT, a0):
            # --- expand + relu + square -----------------------------------
            rbig = px.tile([128, KD, TOK], bf16, tag="rbig", bufs=2)
            ubig = px.tile([128, KD, TOK], bf16, tag="ubig", bufs=2)
            for k in range(KD):
                up = ps_mm.tile([128, TOK], f32, tag="mmout")
                nc.tensor.matmul(up, lhsT=w1e[:, k, :], rhs=a0,
                                 start=True, stop=True)
                nc.scalar.activation(rbig[:, k, :], up, AF.Relu,
                                     bias=biasu[:, k:k + 1], scale=1.0)
                if k % 2 == 0:
                    nc.vector.tensor_mul(ubig[:, k, :], rbig[:, k, :],
                                         rbig[:, k, :])
                else:
                    nc.gpsimd.tensor_mul(ubig[:, k, :], rbig[:, k, :],
                                         rbig[:, k, :])

            # --- comp matmul + centered bias + square ---------------------
            yb = px.tile([128, 4, TOK], bf16, tag="yb", bufs=3,
                         name=f"yb{t}")
            sqy = px.tile([128, 4, TOK], bf16, tag="sqy", bufs=2)
            for j in range(4):
                yp = ps_y.tile([128, TOK], f32, tag="ypre")
                nc.tensor.matmul(yp[0:64, :], lhsT=w2c[:, 2 * j, :],
                                 rhs=ubig[:, 2 * j, :], start=True, stop=True)
                nc.tensor.matmul(yp[64:128, :], lhsT=w2c[:, 2 * j + 1, :],
                                 rhs=ubig[:, 2 * j + 1, :], start=True,
                                 stop=True, tile_position=(0, 64))
                nc.scalar.activation(yb[:, j, :], yp, AF.Identity,
                                     bias=b2f[:, j:j + 1], scale=1.0)
                nc.gpsimd.tensor_mul(sqy[:, j, :], yb[:, j, :], yb[:, j, :])

            # --- per-comp variance via PE; rstd = 1/sqrt(var+eps) ---------
            vst = ps_small.tile([C, TOK], f32, tag="small")
            for j in range(4):
                nc.tensor.matmul(vst, lhsT=vstl[:, j, :], rhs=sqy[:, j, :],
                                 start=(j == 0), stop=(j == 3))
            sd2 = psm.tile([C, TOK], f32, tag="sd2", bufs=2)
            nc.scalar.activation(sd2, vst, AF.Sqrt, bias=ceps, scale=1.0)
            rr = psm.tile([C, TOK], f32, tag="rr", bufs=2)
            if use_recip_approx:
                nc.vector.reciprocal_approx_fast(out=rr, in_=sd2)
            else:
                nc.vector.reciprocal(out=rr, in_=sd2)
            rrb = psm.tile([C, TOK], bf16, tag="rrb", bufs=3, name=f"rrb{t}")
            nc.vector.tensor_copy(out=rrb, in_=rr)
            return xt, yb, rrb

        def stage_back(t, xt, yb, rrb):
            row0 = t * TOK
            # rstd broadcast via selector matmuls; ycT = yb * rstd
            ycT = px.tile([128, 4, TOK], bf16, tag="ycT", bufs=2)
            for j in range(4):
                rbP = ps_mm.tile([128, TOK], f32, tag="mmout")
                nc.tensor.matmul(rbP[0:64, :], lhsT=sel[:, 2 * j, :],
                                 rhs=rrb, start=True, stop=True)
                nc.tensor.matmul(rbP[64:128, :], lhsT=sel[:, 2 * j + 1, :],
                                 rhs=rrb, start=True, stop=True,
                                 tile_position=(0, 64))
                nc.vector.tensor_mul(ycT[:, j, :], yb[:, j, :], rbP)

            # --- proj (operand-swapped -> token-major) + residual ---------
            for cch in range(NCH):
                osb = px.tile([128, D], f32, tag="osb", bufs=3)
                for hf in range(2):
                    ud = ps_mm.tile([128, 512], f32, tag="mmout")
                    for j in range(4):
                        nc.tensor.matmul(
                            ud, lhsT=ycT[:, j, cch * 128:(cch + 1) * 128],
                            rhs=wpf[:, j, hf, :],
                            start=(j == 0),
                            stop=(j == 3 and not use_const))
                    if use_const:
                        nc.tensor.matmul(ud, lhsT=ones1, rhs=cvec[:, hf, :],
                                         start=False, stop=True)
                    nc.vector.tensor_add(
                        osb[:, hf * 512:(hf + 1) * 512], ud,
                        xt[:, cch, hf * 512:(hf + 1) * 512])
                nc.sync.dma_start(
                    out=out_d[row0 + cch * 128: row0 + (cch + 1) * 128, :],
                    in_=osb[:, :])

        fr = {}
        md = {}
        for t in range(ntile + 2):
            if t < ntile:
                fr[t] = stage_front(t)
            if 1 <= t <= ntile:
                xt_, hbT_ = fr.pop(t - 1)
                a0_ = stage_mid_a0(t - 1, xt_, hbT_)
            if t >= 2:
                stage_back(t - 2, *md.pop(t - 2))
            if 1 <= t <= ntile:
                md[t - 1] = stage_mid(t - 1, xt_, hbT_, a0_)

    nc.compile()
    return nc


def _pack_params(anchors, ln_g, W1, b1, W2, b2, cg, cb, Wp, bp, gate,
                 interleaved_t=True):
    f32 = np.float32
    anchors = anchors.astype(f32)
    an = anchors / np.maximum(
        np.linalg.norm(anchors.astype(np.float64), axis=1, keepdims=True),
        1e-12).astype(f32)
    ag = (an * ln_g[None, :].astype(f32)).astype(f32)  # [A, D]

    # agt[p, s, m] = ag[m, d(p,s)]
    agt = np.zeros((128, KD, A), f32)
    dd = np.arange(D)
    if interleaved_t:
        pidx, sidx = dd // KD, dd % KD
    else:
        pidx, sidx = dd % 128, dd // 128
    agt[pidx, sidx, :] = ag.T[dd, :]

    # W1exp[m, f] with m=j*C+k2, f=k*128+e -> value W1[k, j, e] iff k2==k
    W1 = W1.astype(f32)
    w1exp = np.zeros((A, C, E2), f32)
    for m in range(A):
        j, k2 = m // C, m % C
        w1exp[m, k2, :] = W1[k2, j, :]
    w1e = (-w1exp).reshape(A, C, E2)  # [16, 8, 128] (f = k*128+e)
    sf = w1exp.sum(axis=0)  # [C, E2]
    biasu = (sf + b1.astype(f32)).T.copy()  # [128, C] (partition=e, col=k)

    W2 = W2.astype(f32)
    w2m = W2.mean(axis=2, keepdims=True)
    w2cent = W2 - w2m  # [C, E2, DC]
    w2c = np.transpose(w2cent, (1, 0, 2)).copy()  # [128, C, 64]
    b2c = b2.astype(f32) - b2.astype(f32).mean(axis=1, keepdims=True)  # [C, DC]

    b2f = np.zeros((128, 4), f32)
    vstl = np.zeros((128, 4, C), f32)
    for j in range(4):
        for p in range(128):
            kk = 2 * j + p // 64
            b2f[p, j] = b2c[kk, p % 64]
            vstl[p, j, kk] = 1.0 / DC

    sig = (1.0 / (1.0 + np.exp(-gate.astype(np.float64)))).astype(f32)  # [D]
    wpfold = (cg.astype(f32).reshape(C * DC, 1) * Wp.astype(f32)) * sig[None, :]
    wpf = np.ascontiguousarray(
        wpfold.reshape(4, 128, 2, 512).transpose(1, 0, 2, 3))

    const = (cb.astype(f32).reshape(-1) @ Wp.astype(f32) + bp.astype(f32)) * sig
    use_const = bool(np.max(np.abs(const)) > 0)

    import ml_dtypes
    bf16 = ml_dtypes.bfloat16
    sel = np.zeros((C, C, 64), f32)
    for r in range(C):
        sel[r, r, :] = 1.0

    params = dict(
        sel=sel.astype(bf16),
        agt=agt.astype(bf16),
        w1e=w1e.astype(bf16),
        biasu=biasu.astype(f32),
        w2c=w2c.astype(bf16),
        vstl=vstl.astype(bf16),
        b2f=b2f.astype(f32),
        wpf=wpf.astype(bf16),
    )
    if use_const:
        params["cvec"] = const.reshape(1, 2, 512).astype(bf16)
    return params, use_const


def kernel(**inputs):
    x = np.asarray(inputs["x"], dtype=np.float32)
    ln_g = np.asarray(inputs["ln_g"], dtype=np.float32)
    ln_b = np.asarray(inputs["ln_b"], dtype=np.float32)

    fast = (np.allclose(ln_g, 1.0, atol=1e-12) and
            np.allclose(ln_b, 0.0, atol=1e-12))
    if not fast:
        return _np_reference(
            x, *[np.asarray(inputs[k], dtype=np.float32) for k in
                 ("anchors", "ln_g", "ln_b", "W1", "b1", "W2", "b2", "cg",
                  "cb", "Wp", "bp", "gate")])

    params, use_const = _pack_params(
        inputs["anchors"], ln_g, inputs["W1"], inputs["b1"], inputs["W2"],
        inputs["b2"], inputs["cg"], inputs["cb"], inputs["Wp"], inputs["bp"],
        inputs["gate"], interleaved_t=INTERLEAVED_T)

    nc = _build_program(S, use_const, INTERLEAVED_T, USE_RECIP_APPROX)

    from concourse.bass_utils import run_bass_kernel_spmd
    in_maps = []
    for b in range(NCORES):
        m = dict(params)
        m["x"] = np.ascontiguousarray(x[b])
        in_maps.append(m)
    res = run_bass_kernel_spmd(nc, in_maps, core_ids=list(range(NCORES)))
    out = np.stack([res.results[b]["out"] for b in range(NCORES)], axis=0)
    return out.reshape(B, S, D).astype(np.float32)


INTERLEAVED_T = True
USE_RECIP_APPROX = True


# revision 16
# speedup vs baseline: 1.4155x; 1.0108x over previous
"""Trainium2 Bass kernel for nn_ConstellationRelay.

Computation (per token, D=1024, A=16 anchors, C=8 comps, dc=64):
  h   = l2norm(layernorm(x; ln_g, ln_b))
  tri = 1 - h @ l2norm(anchors).T                       (N, 16)
  u   = relu(einsum('nak,kae->nke', tri_g, W1) + b1)^2  (N, 8, 128)
  y   = layernorm_c(u @ W2 + b2; cg, cb)                (N, 8, 64)
  out = x + sigmoid(gate) * (y.flat @ Wp + bp)

Strategy: pure data-parallel over batch (one of 8 NeuronCores per batch row).
On-device fast path requires ln_g==1, ln_b==0 (always true for this problem's
setup_inputs); every other parameter is handled generally via host-side
folding:
  * h = (x - mu)/sqrt(1024*var)  -- eps cancels exactly through the l2norm
  * tri/W1 stage folded into two small matmuls (A0 = a_norm @ h; expand with
    -W1exp and bias (sum_m W1exp + b1) applied in the ReLU activation)
  * comp-LN mean-subtraction folded into centered W2/b2 (host)
  * cg, cb, bp, sigmoid(gate) folded into Wp/const (host)
Layout: token-major for stats/residual, feature-major (via DMA-transpose of
bf16 h) for all matmuls; proj matmul operand-swapped so the residual add
lands token-major in PSUM.
"""

import functools
import os
import sys

import numpy as np

for _p in ("/opt/trn_rl_repo",):
    if _p not in sys.path and os.path.isdir(_p):
        sys.path.insert(0, _p)

B, S, D = 8, 4096, 1024
A, C, DC = 16, 8, 64
APC = A // C  # anchors per compartment
E2 = 2 * DC  # 128, expanded width per comp
NCORES = 8
TOK = 512  # tokens per pipeline tile
NTILE = S // TOK  # 8
NCH = TOK // 128  # 4 token chunks of 128 per tile
KD = D // 128  # 8 feature chunks


def _np_reference(x, anchors, ln_g, ln_b, W1, b1, W2, b2, cg, cb, Wp, bp, gate):
    """Pure-numpy fallback, mirrors reference.py (used only if ln_g/ln_b
    deviate from the values this problem's setup_inputs produces)."""
    x = x.astype(np.float32)
    N = x.shape[0] * x.shape[1]
    xf = x.reshape(N, D)
    mu = xf.mean(-1, keepdims=True)
    var = ((xf - mu) ** 2).mean(-1, keepdims=True)
    h = (xf - mu) / np.sqrt(var + 1e-5) * ln_g + ln_b
    h = h / np.maximum(np.linalg.norm(h, axis=-1, keepdims=True), 1e-12)
    a = anchors / np.maximum(np.linalg.norm(anchors, axis=-1, keepdims=True), 1e-12)
    tri = 1.0 - h @ a.T
    g = tri.reshape(N, APC, C)
    u = np.einsum("nak,kae->nke", g, W1) + b1
    u = np.square(np.maximum(u, 0.0))
    y = np.einsum("nke,ked->nkd", u, W2) + b2
    muy = y.mean(-1, keepdims=True)
    vy = ((y - muy) ** 2).mean(-1, keepdims=True)
    y = (y - muy) / np.sqrt(vy + 1e-5) * cg + cb
    upd = y.reshape(N, C * DC) @ Wp + bp
    sig = 1.0 / (1.0 + np.exp(-gate))
    return (xf + sig * upd).reshape(x.shape).astype(np.float32)


@functools.lru_cache(maxsize=4)
def _build_program(n_tokens=S, use_const=False, interleaved_t=True,
                   use_recip_approx=True):
    """Build + schedule the single-core Bass program (same program runs SPMD
    on all 8 cores).

    interleaved_t: if True, the 3D-output dma_start_transpose writes feature
    d of h to (partition=d//KD, sub=d%KD); host packs the anchor matrix to
    match.  If False, use 32 plain 128x128 transposes with the natural
    d=(chunk*128+p) layout.
    """
    import concourse.bacc as bacc
    import concourse.mybir as mybir
    import concourse.tile as tile

    f32 = mybir.dt.float32
    bf16 = mybir.dt.bfloat16
    AF = mybir.ActivationFunctionType
    OP = mybir.AluOpType

    ntile = n_tokens // TOK

    nc = bacc.Bacc("TRN2", target_bir_lowering=False, debug=False,
                   num_devices=NCORES)

    x_d = nc.dram_tensor("x", [n_tokens, D], f32, kind="ExternalInput")
    agt_d = nc.dram_tensor("agt", [128, KD, 112], bf16, kind="ExternalInput")
    w1e_d = nc.dram_tensor("w1e", [112, KD, 128], bf16, kind="ExternalInput")
    biasu_d = nc.dram_tensor("biasu", [128, KD], f32, kind="ExternalInput")
    w2c_d = nc.dram_tensor("w2c", [128, C, DC], bf16, kind="ExternalInput")
    vstl_d = nc.dram_tensor("vstl", [128, 4, C], bf16, kind="ExternalInput")
    b2f_d = nc.dram_tensor("b2f", [128, 4], f32, kind="ExternalInput")
    wpf_d = nc.dram_tensor("wpf", [128, 4, 2, 512], bf16, kind="ExternalInput")
    sel_d = nc.dram_tensor("sel", [C, 4, 128], bf16, kind="ExternalInput")
    cvec_d = nc.dram_tensor("cvec", [1, 2, 512], bf16, kind="ExternalInput") \
        if use_const else None
    out_d = nc.dram_tensor("out", [n_tokens, D], f32, kind="ExternalOutput")

    from contextlib import ExitStack

    with tile.TileContext(nc) as tc, ExitStack() as ctx:
        pp = ctx.enter_context(tc.tile_pool(name="params", bufs=1))
        agt = pp.tile([128, KD, 112], bf16)
        nc.sync.dma_start(out=agt, in_=agt_d[:, :, :])
        w1e = pp.tile([112, KD, 128], bf16)
        nc.sync.dma_start(out=w1e, in_=w1e_d[:, :, :])
        biasu = pp.tile([128, KD], f32)
        nc.sync.dma_start(out=biasu, in_=biasu_d[:, :])
        w2c = pp.tile([128, C, DC], bf16)
        nc.sync.dma_start(out=w2c, in_=w2c_d[:, :, :])
        vstl = pp.tile([128, 4, C], bf16)
        nc.sync.dma_start(out=vstl, in_=vstl_d[:, :, :])
        b2f = pp.tile([128, 4], f32)
        nc.sync.dma_start(out=b2f, in_=b2f_d[:, :])
        wpf = pp.tile([128, 4, 2, 512], bf16)
        nc.sync.dma_start(out=wpf, in_=wpf_d[:, :, :, :])
        sel = pp.tile([C, 4, 128], bf16)
        nc.sync.dma_start(out=sel, in_=sel_d[:, :, :])
        if use_const:
            cvec = pp.tile([1, 2, 512], bf16)
            nc.sync.dma_start(out=cvec, in_=cvec_d[:, :, :])
            ones1 = pp.tile([1, 128], bf16)
            nc.vector.memset(ones1, 1.0)
        ctiny = pp.tile([128, 1], f32)
        nc.vector.memset(ctiny, 1e-38)
        ceps = pp.tile([C, 1], f32)
        nc.vector.memset(ceps, 1e-5)
        czero = pp.tile([C, 1], f32)
        nc.vector.memset(czero, 0.0)

        px = ctx.enter_context(tc.tile_pool(name="px", bufs=2))
        psm = ctx.enter_context(tc.tile_pool(name="psm", bufs=8))
        # PSUM pools: 2 + 2 + 4 = 8 banks exactly.
        ps_small = ctx.enter_context(tc.tile_pool(name="ps_small", bufs=2,
                                                  space="PSUM"))
        ps_y = ctx.enter_context(tc.tile_pool(name="ps_y", bufs=2,
                                              space="PSUM"))
        ps_mm = ctx.enter_context(tc.tile_pool(name="ps_mm", bufs=4,
                                               space="PSUM"))

        def stage_front(t):
            """Load + stats + normalize + transpose (DMA/DVE/ACT only)."""
            row0 = t * TOK
            xt = px.tile([128, NCH, D], f32, tag="xt", bufs=3, name=f"xt{t}")
            for cch in range(NCH):
                nc.sync.dma_start(
                    out=xt[:, cch, :],
                    in_=x_d[row0 + cch * 128: row0 + (cch + 1) * 128, :])
            hb = px.tile([128, NCH, D], bf16, tag="hb", bufs=2, name=f"hb{t}")
            mv = psm.tile([128, NCH, 2], f32, tag="mv", name=f"mv{t}")
            for cch in range(NCH):
                st = psm.tile([128, 2, 6], f32, tag="st")
                xr = xt[:, cch, :].rearrange("p (s f) -> p s f", s=2)
                nc.vector.bn_stats(out=st[:, 0, :], in_=xr[:, 0, :])
                nc.vector.bn_stats(out=st[:, 1, :], in_=xr[:, 1, :])
                nc.vector.bn_aggr(out=mv[:, cch, :], in_=st)
            sd = psm.tile([128, NCH], f32, tag="sd")
            nc.scalar.activation(sd, mv[:, :, 1], AF.Sqrt, bias=ctiny,
                                 scale=float(D))
            ee = psm.tile([128, NCH], f32, tag="ee", name=f"ee{t}")
            nc.vector.reciprocal(ee, sd)
            bh = psm.tile([128, NCH], f32, tag="bh", name=f"bh{t}")
            nc.vector.scalar_tensor_tensor(
                out=bh, in0=mv[:, :, 0], scalar=-1.0, in1=ee,
                op0=OP.mult, op1=OP.mult)
            for cch in range(NCH):
                nc.scalar.activation(hb[:, cch, :], xt[:, cch, :], AF.Identity,
                                     bias=bh[:, cch:cch + 1],
                                     scale=ee[:, cch:cch + 1])
            hbT = px.tile([128, KD, TOK], bf16, tag="hbT", bufs=2,
                          name=f"hbT{t}")
            if interleaved_t:
                for cch in range(NCH):
                    nc.sync.dma_start_transpose(
                        out=hbT[:, :, cch * 128:(cch + 1) * 128],
                        in_=hb[:, cch, :])
            else:
                for cch in range(NCH):
                    for dch in range(KD):
                        nc.sync.dma_start_transpose(
                            out=hbT[:, dch, cch * 128:(cch + 1) * 128],
                            in_=hb[:, cch, dch * 128:(dch + 1) * 128])
            return xt, hbT

        def stage_mid_a0(t, xt, hbT):
            # --- A0 = a_norm @ h, 4 replicas at partitions {0,32,64,96} ---
            a0p = ps_small.tile([112, TOK], f32, tag="small")
            for dch in range(KD):
                nc.tensor.matmul(a0p, lhsT=agt[:, dch, :], rhs=hbT[:, dch, :],
                                 start=(dch == 0), stop=(dch == KD - 1))
            a0 = psm.tile([112, TOK], bf16, tag="a0", bufs=2)
            nc.scalar.copy(out=a0, in_=a0p)
            return a0

        def stage_mid(t, xt, hbT, a0):
            # --- expand (4-way row-packed) + relu + square ----------------
            rbig = px.tile([128, KD, TOK], bf16, tag="rbig", bufs=2)
            ubig = px.tile([128, KD, TOK], bf16, tag="ubig", bufs=2)
            for kg in range(2):
                ups = []
                for r in range(4):
                    k = 4 * kg + r
                    up = ps_mm.tile([128, TOK], f32, tag="mmout")
                    nc.tensor.matmul(
                        up, lhsT=w1e[32 * r:32 * r + A, k, :],
                        rhs=a0[32 * r:32 * r + A, :],
                        start=True, stop=True,
                        tile_position=(32 * r, 0))
                    ups.append(up)
                for r in range(4):
                    k = 4 * kg + r
                    nc.scalar.activation(rbig[:, k, :], ups[r], AF.Relu,
                                         bias=biasu[:, k:k + 1], scale=1.0)
                    if k % 2 == 0:
                        nc.vector.tensor_mul(ubig[:, k, :], rbig[:, k, :],
                                             rbig[:, k, :])
                    else:
                        nc.gpsimd.tensor_mul(ubig[:, k, :], rbig[:, k, :],
                                             rbig[:, k, :])

            # --- comp matmul + centered bias + square ---------------------
            yb = px.tile([128, 4, TOK], bf16, tag="yb", bufs=3,
                         name=f"yb{t}")
            sqy = px.tile([128, 4, TOK], bf16, tag="sqy", bufs=2)
            for j in range(4):
                yp = ps_y.tile([128, TOK], f32, tag="ypre")
                nc.tensor.matmul(yp[0:64, :], lhsT=w2c[:, 2 * j, :],
                                 rhs=ubig[:, 2 * j, :], start=True, stop=True)
                nc.tensor.matmul(yp[64:128, :], lhsT=w2c[:, 2 * j + 1, :],
                                 rhs=ubig[:, 2 * j + 1, :], start=True,
                                 stop=True, tile_position=(0, 64))
                nc.scalar.activation(yb[:, j, :], yp, AF.Identity,
                                     bias=b2f[:, j:j + 1], scale=1.0)
                nc.gpsimd.tensor_mul(sqy[:, j, :], yb[:, j, :], yb[:, j, :])

            # --- per-comp variance via PE; rstd = 1/sqrt(var+eps) ---------
            vst = ps_small.tile([C, TOK], f32, tag="small")
            for j in range(4):
                nc.tensor.matmul(vst, lhsT=vstl[:, j, :], rhs=sqy[:, j, :],
                                 start=(j == 0), stop=(j == 3))
            sd2 = psm.tile([C, TOK], f32, tag="sd2", bufs=2)
            nc.scalar.activation(sd2, vst, AF.Sqrt, bias=ceps, scale=1.0)
            rr = psm.tile([C, TOK], f32, tag="rr", bufs=2)
            if use_recip_approx:
                nc.vector.reciprocal_approx_fast(out=rr, in_=sd2)
            else:
                nc.vector.reciprocal(out=rr, in_=sd2)
            rrb = psm.tile([C, TOK], bf16, tag="rrb", bufs=3, name=f"rrb{t}")
            nc.vector.tensor_copy(out=rrb, in_=rr)
            return xt, yb, rrb

        def stage_back(t, xt, yb, rrb):
            row0 = t * TOK
            # rstd broadcast via selector matmuls; ycT = yb * rstd
            ycT = px.tile([128, 4, TOK], bf16, tag="ycT", bufs=2)
            for j in range(4):
                rbP = ps_mm.tile([128, TOK], f32, tag="mmout")
                nc.tensor.matmul(rbP, lhsT=sel[:, j, :], rhs=rrb,
                                 start=True, stop=True)
                nc.vector.tensor_mul(ycT[:, j, :], yb[:, j, :], rbP)

            # --- proj (operand-swapped -> token-major) + residual ---------
            for cch in range(NCH):
                osb = px.tile([128, D], f32, tag="osb", bufs=3)
                for hf in range(2):
                    ud = ps_mm.tile([128, 512], f32, tag="mmout")
                    for j in range(4):
                        nc.tensor.matmul(
                            ud, lhsT=ycT[:, j, cch * 128:(cch + 1) * 128],
                            rhs=wpf[:, j, hf, :],
                            start=(j == 0),
                            stop=(j == 3 and not use_const))
                    if use_const:
                        nc.tensor.matmul(ud, lhsT=ones1, rhs=cvec[:, hf, :],
                                         start=False, stop=True)
                    nc.vector.tensor_add(
                        osb[:, hf * 512:(hf + 1) * 512], ud,
                        xt[:, cch, hf * 512:(hf + 1) * 512])
                nc.sync.dma_start(
                    out=out_d[row0 + cch * 128: row0 + (cch + 1) * 128, :],
                    in_=osb[:, :])

        fr = {}
        md = {}
        for t in range(ntile + 2):
            if t < ntile:
                fr[t] = stage_front(t)
            if 1 <= t <= ntile:
                xt_, hbT_ = fr.pop(t - 1)
                a0_ = stage_mid_a0(t - 1, xt_, hbT_)
            if t >= 2:
                stage_back(t - 2, *md.pop(t - 2))
            if 1 <= t <= ntile:
                md[t - 1] = stage_mid(t - 1, xt_, hbT_, a0_)

    nc.compile()
    return nc


def _pack_params(anchors, ln_g, W1, b1, W2, b2, cg, cb, Wp, bp, gate,
                 interleaved_t=True):
    f32 = np.float32
    anchors = anchors.astype(f32)
    an = anchors / np.maximum(
        np.linalg.norm(anchors.astype(np.float64), axis=1, keepdims=True),
        1e-12).astype(f32)
    ag = (an * ln_g[None, :].astype(f32)).astype(f32)  # [A, D]

    # agt[p, s, 32r+m] = ag[m, d(p,s)] for r in 0..3 (4 replicas)
    agt = np.zeros((128, KD, 112), f32)
    dd = np.arange(D)
    if interleaved_t:
        pidx, sidx = dd // KD, dd % KD
    else:
        pidx, sidx = dd % 128, dd // 128
    for r in range(4):
        agt[pidx, sidx, 32 * r:32 * r + A] = ag.T[dd, :]

    # W1exp[m, f] with m=j*C+k2, f=k*128+e -> value W1[k, j, e] iff k2==k
    W1 = W1.astype(f32)
    w1exp = np.zeros((A, C, E2), f32)
    for m in range(A):
        j, k2 = m // C, m % C
        w1exp[m, k2, :] = W1[k2, j, :]
    w1e16 = (-w1exp).reshape(A, C, E2)  # [16, 8, 128] (f = k*128+e)
    w1e = np.zeros((112, C, E2), f32)
    for r in range(4):
        w1e[32 * r:32 * r + A] = w1e16
    sf = w1exp.sum(axis=0)  # [C, E2]
    biasu = (sf + b1.astype(f32)).T.copy()  # [128, C] (partition=e, col=k)

    W2 = W2.astype(f32)
    w2m = W2.mean(axis=2, keepdims=True)
    w2cent = W2 - w2m  # [C, E2, DC]
    w2c = np.transpose(w2cent, (1, 0, 2)).copy()  # [128, C, 64]
    b2c = b2.astype(f32) - b2.astype(f32).mean(axis=1, keepdims=True)  # [C, DC]

    b2f = np.zeros((128, 4), f32)
    vstl = np.zeros((128, 4, C), f32)
    for j in range(4):
        for p in range(128):
            kk = 2 * j + p // 64
            b2f[p, j] = b2c[kk, p % 64]
            vstl[p, j, kk] = 1.0 / DC

    sig = (1.0 / (1.0 + np.exp(-gate.astype(np.float64)))).astype(f32)  # [D]
    wpfold = (cg.astype(f32).reshape(C * DC, 1) * Wp.astype(f32)) * sig[None, :]
    wpf = np.ascontiguousarray(
        wpfold.reshape(4, 128, 2, 512).transpose(1, 0, 2, 3))

    const = (cb.astype(f32).reshape(-1) @ Wp.astype(f32) + bp.astype(f32)) * sig
    use_const = bool(np.max(np.abs(const)) > 0)

    import ml_dtypes
    bf16 = ml_dtypes.bfloat16
    sel = np.zeros((C, 4, 128), f32)
    for j in range(4):
        sel[2 * j, j, 0:64] = 1.0
        sel[2 * j + 1, j, 64:128] = 1.0

    params = dict(
        sel=sel.astype(bf16),
        agt=agt.astype(bf16),
        w1e=w1e.astype(bf16),
        biasu=biasu.astype(f32),
        w2c=w2c.astype(bf16),
        vstl=vstl.astype(bf16),
        b2f=b2f.astype(f32),
        wpf=wpf.astype(bf16),
    )
    if use_const:
        params["cvec"] = const.reshape(1, 2, 512).astype(bf16)
    return params, use_const


def kernel(**inputs):
    x = np.asarray(inputs["x"], dtype=np.float32)
    ln_g = np.asarray(inputs["ln_g"], dtype=np.float32)
    ln_b = np.asarray(inputs["ln_b"], dtype=np.float32)

    fast = (np.allclose(ln_g, 1.0, atol=1e-12) and
            np.allclose(ln_b, 0.0, atol=1e-12))
    if not fast:
        return _np_reference(
            x, *[np.asarray(inputs[k], dtype=np.float32) for k in
                 ("anchors", "ln_g", "ln_b", "W1", "b1", "W2", "b2", "cg",
                  "cb", "Wp", "bp", "gate")])

    params, use_const = _pack_params(
        inputs["anchors"], ln_g, inputs["W1"], inputs["b1"], inputs["W2"],
        inputs["b2"], inputs["cg"], inputs["cb"], inputs["Wp"], inputs["bp"],
        inputs["gate"], interleaved_t=INTERLEAVED_T)

    nc = _build_program(S, use_const, INTERLEAVED_T, USE_RECIP_APPROX)

    from concourse.bass_utils import run_bass_kernel_spmd
    in_maps = []
    for b in range(NCORES):
        m = dict(params)
        m["x"] = np.ascontiguousarray(x[b])
        in_maps.append(m)
    res = run_bass_kernel_spmd(nc, in_maps, core_ids=list(range(NCORES)))
    out = np.stack([res.results[b]["out"] for b in range(NCORES)], axis=0)
    return out.reshape(B, S, D).astype(np.float32)


INTERLEAVED_T = True
USE_RECIP_APPROX = True
